# revision 47
# baseline (speedup 1.0000x reference)
"""EntityAggrNet (2-layer GNN message passing) on 8 Trainium2 NeuronCores.

Strategy
--------
Node-parallel sharding: core w owns nodes [w*2048, (w+1)*2048).  Edges are
sorted by src on the host; each core processes the edges whose src lands in
its node range (edge counts per core are within ~1% of E/8 for random edges).

Per layer, per core:
  1. dma_gather x[dst] rows (bf16, 512B rows) from a replicated HBM copy of
     the layer input, 1024 edges per gather call, spread over 4 SWDGE queues.
  2. Segment-sum via one-hot selector matmuls: for each 128-edge chunk,
     selector[p, j] = (src[p] - window_base == j) built with a DVE is_equal
     against an iota row (one batched build per gather piece); PE accumulates
     selector.T @ gathered into a PSUM window of 128 nodes.  Mean = PSUM *
     (1/cnt) on evacuation.
  3. Linearity moves the weight matmuls out of the edge loop:
     mean(x[dst]) @ W_msg.  The edge-feature path collapses to
     Hn @ (emb_table @ W_edge) where Hn[n, d] = count(src=n, feat=d)/cnt[n]
     is a host-computed *index* matrix; biases ride along as extra Hn rows.
  4. Dense phase in feature-major layout (PE transposes), BatchNorm stats
     via a 2KB AllReduce; BN + ReLU applied feature-major on the scalar
     engine (per-partition scale/shift) — no node-major BN pass.
  5. Layer-1 output is PE-transposed to node-major bf16 and AllGathered to
     become layer 2's gather source.  The final output stays feature-major
     [D, NPC]; the host transposes during unsharding.

Edge path (gathered x, selectors) runs in bf16; dense path in float32r.
"""
import os
import sys

if "/opt/trn_rl_repo" not in sys.path:
    sys.path.insert(0, "/opt/trn_rl_repo")

import numpy as np

import concourse.bass as bass  # noqa: F401  (engine types referenced via nc)
import concourse.tile as tile
from concourse import bacc, mybir
from concourse import bass_utils
from concourse.bass_interp import get_hw_module

F32 = mybir.dt.float32
F32R = mybir.dt.float32r
I16 = mybir.dt.int16
BF16 = mybir.dt.bfloat16
ALU = mybir.AluOpType
ACTF = mybir.ActivationFunctionType

EPS = 1e-5
B, S, D = 32, 512, 256
N = B * S                # 16384 nodes
DS, DD = 64, 64          # embedding table: [DS, DD]
L = 2                    # layers
NCORE = 8
NPC = N // NCORE         # 2048 nodes per core
WIN = 128                # nodes per PSUM window
NWIN = NPC // WIN        # 16 windows per core
NWING = N // WIN         # 128 windows globally
_CACHE = {}


def _pick_piece(epc):
    """Largest 128-multiple piece that divides epc and fits the SWDGE ring
    (descs/engine = piece/16 + 1 <= 128 -> piece <= 2032)."""
    for piece in range(1024, 512, -128):
        if epc % piece == 0:
            return piece
    return 512


def _balance(cnt):
    """Assign nodes to (window, pos) so per-window edge load is near-uniform.

    Greedy LPT over the 128 global windows (16 per core, 128 nodes each).
    Returns perm (node -> global position) and nch (chunks per window).
    Index-only computation.
    """
    import heapq
    order = np.argsort(-cnt, kind="stable")
    fill = np.zeros(NWING, np.int64)
    loads = np.zeros(NWING, np.int64)
    perm = np.empty(N, np.int64)
    heap = [(0, w) for w in range(NWING)]
    heapq.heapify(heap)
    for n in order:
        while True:
            load, w = heapq.heappop(heap)
            if fill[w] < WIN:
                break
        perm[n] = w * WIN + fill[w]
        fill[w] += 1
        loads[w] = load + cnt[n]
        if fill[w] < WIN:
            heapq.heappush(heap, (int(loads[w]), w))
    nch = max(int(np.ceil(loads.max() / 128)), 1)
    return perm, nch


def _build(nch, piece_src):
    """Build + schedule + bacc-compile the SPMD program.

    nch: chunks (of 128 edges) per 128-node window, uniform across cores
    (host pads every window to nch*128 edges).
    piece_src: per gather piece, 0 if every slot's dst lands in the first
    AllGather half (so the piece may start after AG#0), else 1.
    """
    cap = nch * WIN                  # padded edges per window
    epc = NWIN * cap                 # padded edges per core
    PIECE = _pick_piece(epc)         # edges per dma_gather call
    npiece = epc // PIECE            # gather calls per layer
    assert epc % PIECE == 0 and nch >= 2
    assert len(piece_src) == npiece
    NPCH = PIECE // 128              # chunks per piece

    nc = bacc.Bacc("TRN2", target_bir_lowering=False, debug=False,
                   num_devices=NCORE, num_swdge_queues=4)

    # ---- I/O ----
    x0 = nc.dram_tensor("x0", [N, D], BF16, kind="ExternalInput")
    xT0 = nc.dram_tensor("xT0", [D, NPC], F32R, kind="ExternalInput")
    idx_in = nc.dram_tensor("idx_in", [128, epc // 16], I16, kind="ExternalInput")
    srcmod_in = nc.dram_tensor("srcmod_in", [128, NWIN * nch], BF16, kind="ExternalInput")
    recip_in = nc.dram_tensor("recip_in", [128, NWIN], F32, kind="ExternalInput")
    hnt_in = nc.dram_tensor("hnt_in", [128, NPC], F32R, kind="ExternalInput")
    iota_in = nc.dram_tensor("iota_in", [128, 128], BF16, kind="ExternalInput")
    ident_in = nc.dram_tensor("ident_in", [128, 128], F32, kind="ExternalInput")
    identr_in = nc.dram_tensor("identr_in", [128, 128], F32R, kind="ExternalInput")
    wm_in = [nc.dram_tensor(f"wm{l}", [D, D], F32R, kind="ExternalInput") for l in range(L)]
    ws_in = [nc.dram_tensor(f"ws{l}", [D, D], F32R, kind="ExternalInput") for l in range(L)]
    embT_in = nc.dram_tensor("embT", [DD, DS], F32R, kind="ExternalInput")
    we_in = [nc.dram_tensor(f"we{l}", [DD, D], F32R, kind="ExternalInput") for l in range(L)]
    # rows DD..127 of the EW lhsT: [bm, be, bs, zeros...] packed on host
    ewc_in = [nc.dram_tensor(f"ewc{l}", [128 - DD, D], F32R, kind="ExternalInput")
              for l in range(L)]
    gam_in = [nc.dram_tensor(f"gam{l}", [D, 1], F32, kind="ExternalInput") for l in range(L)]
    bet_in = [nc.dram_tensor(f"bet{l}", [D, 1], F32, kind="ExternalInput") for l in range(L)]
    out_ext = nc.dram_tensor("out", [D, NPC], F32, kind="ExternalOutput")

    with tile.TileContext(nc) as tc:
        with tc.tile_pool(name="const", bufs=1) as cp, \
             tc.tile_pool(name="gat", bufs=4) as gp, \
             tc.tile_pool(name="selp", bufs=4) as sp, \
             tc.tile_pool(name="xmaj", bufs=1) as xp, \
             tc.tile_pool(name="psE", bufs=4, space="PSUM") as psE, \
             tc.tile_pool(name="psT", bufs=2, space="PSUM") as psT, \
             tc.tile_pool(name="psD", bufs=2, space="PSUM") as psD, \
             tc.tile_pool(name="dram", bufs=1, space="DRAM") as dp:

            # ---- constants into SBUF ----
            # Gather/selector gating inputs load first so the edge phase
            # starts as early as possible.
            NA_PIECES = 4                    # pieces preloaded first
            CPP = PIECE // 16                # idx columns per piece
            NSPLIT = NA_PIECES * CPP
            idx_a = cp.tile([128, NSPLIT], I16)
            idx_b = cp.tile([128, epc // 16 - NSPLIT], I16)
            srcmod = cp.tile([128, NWIN * nch], BF16)
            recip = cp.tile([128, NWIN], F32)
            hnt = cp.tile([128, NPC], F32R)
            iota = cp.tile([128, 128], BF16)
            ident = cp.tile([128, 128], F32)
            identr = cp.tile([128, 128], F32R)
            nc.sync.dma_start(out=idx_a[:, :], in_=idx_in[:, 0:NSPLIT])
            nc.sync.dma_start(out=iota[:, :], in_=iota_in[:, :])
            nc.sync.dma_start(out=srcmod[:, :], in_=srcmod_in[:, :])
            nc.sync.dma_start(out=idx_b[:, :], in_=idx_in[:, NSPLIT:])
            nc.sync.dma_start(out=recip[:, :], in_=recip_in[:, :])
            nc.sync.dma_start(out=ident[:, :], in_=ident_in[:, :])
            nc.sync.dma_start(out=identr[:, :], in_=identr_in[:, :])
            # big constants ride other engines' HWDGE queues so they don't
            # delay the idx/selector loads gating the first gathers
            nc.scalar.dma_start(out=hnt[:, :], in_=hnt_in[:, :])

            wm_sb, ws_sb, ew_sb = [], [], []
            embT_sb = cp.tile([DD, DS], F32R)
            nc.sync.dma_start(out=embT_sb[:, :], in_=embT_in[:, :])
            for l in range(L):
                wm = cp.tile([128, 2, D], F32R, name=f"wm_sb{l}")
                ws = cp.tile([128, 2, D], F32R, name=f"ws_sb{l}")
                for kt in range(2):
                    nc.scalar.dma_start(out=wm[:, kt, :], in_=wm_in[l][kt * 128:(kt + 1) * 128, :])
                    nc.scalar.dma_start(out=ws[:, kt, :], in_=ws_in[l][kt * 128:(kt + 1) * 128, :])
                wm_sb.append(wm)
                ws_sb.append(ws)

                we = cp.tile([DD, D], F32R, name=f"we_sb{l}")
                nc.sync.dma_start(out=we[:, :], in_=we_in[l][:, :])
                psew = psT.tile([DD, D], F32, tag="pst", name=f"psew{l}")
                nc.tensor.matmul(psew[:, :], embT_sb[:, :], we[:, :], start=True, stop=True)
                ew = cp.tile([128, D], F32R, name=f"ew_sb{l}")
                nc.vector.tensor_copy(ew[0:DD, :], psew[:, :])
                nc.sync.dma_start(out=ew[DD:128, :], in_=ewc_in[l][:, :])
                ew_sb.append(ew)

            gb_sb = []  # [128, 2] gamma / beta per layer, packed per feat-half
            for l in range(L):
                gam = cp.tile([128, 2], F32, name=f"gam_sb{l}")
                bet = cp.tile([128, 2], F32, name=f"bet_sb{l}")
                for f in range(2):
                    nc.sync.dma_start(out=gam[:, f:f + 1], in_=gam_in[l][f * 128:(f + 1) * 128, :])
                    nc.sync.dma_start(out=bet[:, f:f + 1], in_=bet_in[l][f * 128:(f + 1) * 128, :])
                gb_sb.append((gam, bet))

            xT_cur = [cp.tile([128, NPC], F32R, name=f"xT0_sb{f}") for f in range(2)]
            for f in range(2):
                nc.scalar.dma_start(out=xT_cur[f][:, :], in_=xT0[f * 128:(f + 1) * 128, :])

            # absorb one-time collective setup cost under the edge phase
            warm_sb = cp.tile([128, 1], F32, name="warm_sb")
            nc.vector.memset(warm_sb[:, :], 0.0)
            warm_in = dp.tile([128, 1], F32, name="warm_in")
            warm_out = dp.tile([128, 1], F32, addr_space="Shared", name="warm_out")
            nc.sync.dma_start(out=warm_in[:, :], in_=warm_sb[:, :])
            nc.gpsimd.collective_compute(
                "AllReduce", ALU.add,
                replica_groups=[list(range(NCORE))],
                ins=[warm_in[:, :]], outs=[warm_out[:, :]])
            warm_bk = cp.tile([128, 1], F32, name="warm_bk")
            nc.sync.dma_start(out=warm_bk[:, :], in_=warm_out[:, :])

            xsrc = x0  # gather source (DRAM AP-able handle)

            for l in range(L):
                # ================= edge phase =================
                pieces = [None] * npiece

                def ensure_piece(p, l=l, pieces=pieces):
                    if pieces[p] is not None:
                        return pieces[p]
                    g = gp.tile([128, NPCH, D], BF16, tag="g", name=f"g{l}_{p}")
                    idx_ap = (idx_a[:, p * CPP:(p + 1) * CPP]
                              if p < NA_PIECES else
                              idx_b[:, (p - NA_PIECES) * CPP:
                                       (p - NA_PIECES + 1) * CPP])
                    nc.gpsimd.dma_gather(
                        out_ap=g[:, :, :],
                        in_ap=xsrc[:, :],
                        idxs_ap=idx_ap,
                        num_idxs=PIECE, num_idxs_reg=PIECE,
                        elem_size=D, single_packet=True,
                        queue_num=p % 4)
                    s = sp.tile([128, NPCH, 128], BF16, tag="s", name=f"s{l}_{p}")
                    nc.vector.tensor_tensor(
                        s[:, :, :],
                        iota[:, :].unsqueeze(1).to_broadcast((128, NPCH, 128)),
                        srcmod[:, p * NPCH:(p + 1) * NPCH].unsqueeze(2)
                              .to_broadcast((128, NPCH, 128)),
                        ALU.is_equal)
                    pieces[p] = (g, s)
                    return pieces[p]

                # Fused edge + dense pipeline: windows stream through; after
                # every 4th window the corresponding 512-node dense block,
                # its stat partials, and its feat-major transposes fire.
                msx = xp.tile([128, NWIN, D], F32, tag="msx", name=f"msx{l}")
                preout = [xp.tile([128, NPC], F32, tag=f"pre{f}", name=f"pre{l}_{f}")
                          for f in range(2)]
                redp = cp.tile([128, 16], F32, tag="redp", bufs=2, name=f"redp{l}")
                sqscr = xp.tile([128, 512], F32, tag="sqscr", name=f"sqscr{l}")
                msxTn = [[None] * (NPC // 512) for _ in range(2)]
                for w in range(NWIN):
                    ps = psE.tile([128, D], F32, tag="pse", name=f"pse{l}_{w}")
                    for c in range(nch):
                        gc = w * nch + c
                        g, s = ensure_piece(gc // NPCH)
                        lc = gc % NPCH
                        nc.tensor.matmul(ps[:, :], s[:, lc, :], g[:, lc, :],
                                         start=(c == 0), stop=(c == nch - 1))
                    nc.vector.tensor_scalar(msx[:, w, :], ps[:, :],
                                            recip[:, w:w + 1], None, ALU.mult)
                    nb, wi = w // 4, w % 4
                    for f in range(2):
                        if wi == 0:
                            msxTn[f][nb] = xp.tile([128, 512], F32R, tag=f"msxT{f}",
                                                   bufs=4, name=f"msxT{l}_{f}_{nb}")
                        pt = psT.tile([128, 128], F32, tag="pst", name=f"ptm{l}_{w}_{f}")
                        nc.tensor.transpose(pt[:, :], msx[:, w, f * 128:(f + 1) * 128],
                                            ident[:, :])
                        dstp = msxTn[f][nb][:, wi * 128:(wi + 1) * 128]
                        if f == 0:
                            nc.vector.tensor_copy(dstp, pt[:, :])
                        else:
                            nc.scalar.activation(dstp, pt[:, :], ACTF.Copy,
                                                 bias=0.0, scale=1.0)
                    if wi != 3:
                        continue
                    # dense block for this group of 4 windows
                    cols = slice(nb * 512, (nb + 1) * 512)
                    for f in range(2):
                        pd = psD.tile([128, 512], F32, tag="psd", name=f"pd{l}_{f}_{nb}")
                        fo = slice(f * 128, (f + 1) * 128)
                        nc.tensor.matmul(pd[:, :], wm_sb[l][:, 0, fo], msxTn[0][nb][:, :],
                                         start=True, stop=False)
                        nc.tensor.matmul(pd[:, :], wm_sb[l][:, 1, fo], msxTn[1][nb][:, :],
                                         start=False, stop=False)
                        nc.tensor.matmul(pd[:, :], ws_sb[l][:, 0, fo], xT_cur[0][:, cols],
                                         start=False, stop=False)
                        nc.tensor.matmul(pd[:, :], ws_sb[l][:, 1, fo], xT_cur[1][:, cols],
                                         start=False, stop=False)
                        nc.tensor.matmul(pd[:, :], ew_sb[l][:, fo], hnt[:, cols],
                                         start=False, stop=True)
                        # evacuate + free per-block column sums
                        nc.vector.tensor_scalar(preout[f][:, cols], pd[:, :],
                                                1.0, 0.0, ALU.mult, ALU.add,
                                                accum_out=redp[:, f * 4 + nb:f * 4 + nb + 1])
                        # per-block sum of squares on the scalar engine
                        nc.scalar.activation(sqscr[:, :], preout[f][:, cols],
                                             ACTF.Square, bias=0.0, scale=1.0,
                                             accum_out=redp[:, 8 + f * 4 + nb:
                                                            9 + f * 4 + nb])

                # ================= batchnorm stats =================
                red = cp.tile([128, 4], F32, tag="red", bufs=2, name=f"red{l}")
                for f in range(2):
                    nc.vector.tensor_reduce(red[:, f:f + 1], redp[:, f * 4:(f + 1) * 4],
                                            mybir.AxisListType.X, ALU.add)
                    nc.vector.tensor_reduce(red[:, 2 + f:3 + f],
                                            redp[:, 8 + f * 4:8 + (f + 1) * 4],
                                            mybir.AxisListType.X, ALU.add)

                st_in = dp.tile([128, 4], F32, name=f"st_in{l}")
                st_out = dp.tile([128, 4], F32, addr_space="Shared", name=f"st_out{l}")
                nc.scalar.dma_start(out=st_in[:, :], in_=red[:, :])
                nc.gpsimd.collective_compute(
                    "AllReduce", ALU.add,
                    replica_groups=[list(range(NCORE))],
                    ins=[st_in[:, :]], outs=[st_out[:, :]])
                red2 = cp.tile([128, 4], F32, tag="red", bufs=2, name=f"red2{l}")
                nc.sync.dma_start(out=red2[:, :], in_=st_out[:, :])

                # mu/var -> scale/shift  (all [128, 2], column form)
                mo = cp.tile([128, 12], F32, tag="mo", bufs=2, name=f"mo{l}")
                mu, ex2, var, vare, sd, rsq = (mo[:, 0:2], mo[:, 2:4], mo[:, 4:6],
                                               mo[:, 6:8], mo[:, 8:10], mo[:, 10:12])
                nc.vector.tensor_scalar(mu, red2[:, 0:2], 1.0 / N, None, ALU.mult)
                nc.vector.tensor_scalar(ex2, red2[:, 2:4], 1.0 / N, None, ALU.mult)
                nc.vector.tensor_tensor(var, mu, mu, ALU.mult)
                nc.vector.tensor_tensor(var, ex2, var, ALU.subtract)
                nc.vector.tensor_scalar(vare, var, EPS, None, ALU.add)
                nc.scalar.activation(sd, vare, ACTF.Sqrt, bias=0.0, scale=1.0)
                nc.vector.reciprocal(rsq, sd)
                gam, bet = gb_sb[l]
                sc = cp.tile([128, 4], F32, tag="sc", bufs=2, name=f"sc{l}")
                scale2, shift2 = sc[:, 0:2], sc[:, 2:4]
                nc.vector.tensor_tensor(scale2, gam[:, :], rsq, ALU.mult)
                nc.vector.tensor_tensor(shift2, mu, scale2, ALU.mult)
                nc.vector.tensor_tensor(shift2, bet[:, :], shift2, ALU.subtract)

                # ===== BN + ReLU feature-major on the scalar engine =====
                xnT = [xp.tile([128, NPC], F32R if l < L - 1 else F32,
                               tag=f"xnT{f}", name=f"xnT{l}_{f}")
                       for f in range(2)]
                for f in range(2):
                    nc.scalar.activation(xnT[f][:, :], preout[f][:, :], ACTF.Relu,
                                         bias=shift2[:, f:f + 1],
                                         scale=scale2[:, f:f + 1])

                if l == L - 1:
                    for f in range(2):
                        nc.sync.dma_start(out=out_ext[f * 128:(f + 1) * 128, :],
                                          in_=xnT[f][:, :])
                else:
                    # node-major bf16 copy for the next layer's gather source
                    agi = dp.tile([NPC, D], BF16, name=f"agi{l}")
                    ago = dp.tile([N, D], BF16, addr_space="Shared", name=f"ago{l}")
                    xrow = xp.tile([128, NWIN, D], BF16, tag="xrow", name=f"xrow{l}")
                    for w in range(NWIN):
                        for f in range(2):
                            pt = psT.tile([128, 128], F32R, tag="pst",
                                          name=f"ptx{l}_{w}_{f}")
                            nc.tensor.transpose(pt[:, :],
                                                xnT[f][:, w * 128:(w + 1) * 128],
                                                identr[:, :])
                            dstp = xrow[:, w, f * 128:(f + 1) * 128]
                            if f == 0:
                                nc.vector.tensor_copy(dstp, pt[:, :])
                            else:
                                nc.scalar.activation(dstp, pt[:, :], ACTF.Copy,
                                                     bias=0.0, scale=1.0)
                    dst_ap = agi[:, :].rearrange("(w p) d -> p w d", p=128)
                    H = NWIN // 2
                    for h in range(2):
                        ws_ = slice(h * H, (h + 1) * H)
                        nc.sync.dma_start(out=dst_ap[:, ws_, :], in_=xrow[:, ws_, :])
                    nc.gpsimd.collective_compute(
                        "AllGather", ALU.bypass,
                        replica_groups=[list(range(NCORE))],
                        ins=[agi[:, :]], outs=[ago[:, :]])
                    xsrc = ago
                    xT_cur = xnT

    nc.compile()
    nc.m = get_hw_module(nc.m)
    return nc


def _preprocess(data, edge, edge_feature):
    """Host-side index preprocessing: balance nodes over windows, sort edges
    by (permuted) src, window-pad, build count matrices.  Touches only index
    arrays (+ dtype/layout of data)."""
    src = np.asarray(edge[0], dtype=np.int64)
    dst = np.asarray(edge[1], dtype=np.int64)
    ef = np.asarray(edge_feature, dtype=np.int64)

    cnt0 = np.bincount(src, minlength=N)
    perm, nch = _balance(cnt0)
    invperm = np.empty(N, np.int64)
    invperm[perm] = np.arange(N)

    psrc = perm[src]
    pdst = perm[dst]
    order = np.argsort(psrc, kind="stable")
    src_s = psrc[order]
    dst_s = pdst[order]

    cnt = np.bincount(psrc, minlength=N)
    recip = (1.0 / np.maximum(cnt, 1)).astype(np.float32)
    H = np.bincount(psrc * DS + ef, minlength=N * DS).reshape(N, DS)
    Hn = (H * recip[:, None]).astype(np.float32)

    cap = nch * WIN
    wcnt = np.bincount(src_s // WIN, minlength=NWING)
    assert wcnt.max() <= cap

    wstart = np.zeros(NWING + 1, np.int64)
    np.cumsum(wcnt, out=wstart[1:])
    idx_pad = np.zeros((NWING, cap), np.int16)   # layer-1 idx (x0 rows)
    idx2_pad = np.zeros((NWING, cap), np.int16)  # layer-2 idx (ago layout)
    srm_pad = np.full((NWING, cap), -1.0, np.float32)
    CH = NPC // 2
    kk_pad = np.zeros((NWING, cap), np.int8)     # AllGather half of each slot
    # ago position of node position p: half-major, then core, then offset
    pco = np.arange(N)
    ago_pos = ((pco % NPC) // CH) * (N // 2) + (pco // NPC) * CH + (pco % CH)
    for g in range(NWING):
        a, b = wstart[g], wstart[g + 1]
        k = b - a
        dsts = dst_s[a:b]
        ordh = np.argsort((dsts % NPC) // CH, kind="stable")
        dsts = dsts[ordh]
        idx_pad[g, :k] = dsts.astype(np.int16)
        idx2_pad[g, :k] = ago_pos[dsts].astype(np.int16)
        srm_pad[g, :k] = (src_s[a:b][ordh] - g * WIN).astype(np.float32)
        kk_pad[g, :k] = ((dsts % NPC) // CH).astype(np.int8)

    piece = _pick_piece(NWIN * cap)
    npiece = NWIN * cap // piece
    piece_src = np.zeros(npiece, np.int8)
    for w in range(NCORE):
        flat_kk = kk_pad[w * NWIN:(w + 1) * NWIN].reshape(-1)
        np.maximum(piece_src, flat_kk.reshape(npiece, piece).max(axis=1),
                   out=piece_src)
    piece_src = tuple(int(v) for v in piece_src)

    data2 = data.reshape(N, D)
    per_core = []
    for w in range(NCORE):
        gsl = slice(w * NWIN, (w + 1) * NWIN)
        nsl = slice(w * NPC, (w + 1) * NPC)
        orig = invperm[nsl]                           # original node ids
        flat_idx = idx_pad[gsl].reshape(-1)           # [NWIN*cap]
        idx_tile = np.tile(flat_idx.reshape(-1, 16).T, (8, 1)).astype(np.int16)

        srcmod = srm_pad[gsl].reshape(-1, 128).T.copy()      # [128, NWIN*nch]
        recip_sw = recip[nsl].reshape(NWIN, 128).T.copy()    # [128, NWIN]
        hnt = np.zeros((128, NPC), np.float32)
        hnt[:DS, :] = Hn[nsl].T
        nz = (cnt[nsl] > 0).astype(np.float32)
        hnt[DS, :] = nz
        hnt[DS + 1, :] = nz
        hnt[DS + 2, :] = 1.0
        xT0 = np.ascontiguousarray(data2[orig].T.astype(np.float32))
        import ml_dtypes as _md
        per_core.append(dict(idx_in=idx_tile,
                             srcmod_in=srcmod.astype(_md.bfloat16),
                             recip_in=recip_sw, hnt_in=hnt, xT0=xT0))
    return nch, perm, invperm, piece_src, per_core


def kernel(data, emb_table, W_msg, b_msg, W_self, b_self, W_edge, b_edge,
           bn_gamma, bn_beta, edge, edge_feature):
    data = np.asarray(data)
    nch, perm, invperm, piece_src, per_core = _preprocess(
        data, np.asarray(edge), np.asarray(edge_feature))

    key = (nch, piece_src)
    if key not in _CACHE:
        _CACHE[key] = _build(nch, piece_src)
    nc = _CACHE[key]

    import ml_dtypes
    x0 = np.ascontiguousarray(
        data.reshape(N, D)[invperm].astype(ml_dtypes.bfloat16))
    iota = np.broadcast_to(np.arange(128), (128, 128)).astype(ml_dtypes.bfloat16)
    ident = np.eye(128, dtype=np.float32)
    common = {
        "x0": x0, "iota_in": iota, "ident_in": ident, "identr_in": ident,
        "embT": np.ascontiguousarray(np.asarray(emb_table, np.float32).T),
    }
    for l in range(L):
        common[f"wm{l}"] = np.ascontiguousarray(np.asarray(W_msg[l], np.float32))
        common[f"ws{l}"] = np.ascontiguousarray(np.asarray(W_self[l], np.float32))
        common[f"we{l}"] = np.ascontiguousarray(np.asarray(W_edge[l], np.float32))
        ewc = np.zeros((128 - DD, D), np.float32)
        ewc[0] = np.asarray(b_msg[l], np.float32)
        ewc[1] = np.asarray(b_edge[l], np.float32)
        ewc[2] = np.asarray(b_self[l], np.float32)
        common[f"ewc{l}"] = ewc
        common[f"gam{l}"] = np.asarray(bn_gamma[l], np.float32).reshape(D, 1)
        common[f"bet{l}"] = np.asarray(bn_beta[l], np.float32).reshape(D, 1)

    in_maps = [{**common, **pc} for pc in per_core]
    trace = bool(os.environ.get("GNN_TRN_TRACE"))
    res = bass_utils.run_bass_kernel_spmd(
        nc, in_maps, core_ids=list(range(NCORE)), trace=trace)
    if trace:
        global LAST_RESULT
        LAST_RESULT = res
    out = np.concatenate([res.results[c]["out"] for c in range(NCORE)], axis=1)
    return np.ascontiguousarray(out.T[perm]).reshape(B, S, D).astype(np.float32)


LAST_RESULT = None


# revision 48
# speedup vs baseline: 1.3886x; 1.3886x over previous
"""EntityAggrNet (2-layer GNN message passing) on 8 Trainium2 NeuronCores.

Strategy
--------
Node-parallel sharding: core w owns nodes [w*2048, (w+1)*2048).  Edges are
sorted by src on the host; each core processes the edges whose src lands in
its node range (edge counts per core are within ~1% of E/8 for random edges).

Per layer, per core:
  1. dma_gather x[dst] rows (bf16, 512B rows) from a replicated HBM copy of
     the layer input, 1024 edges per gather call, spread over 4 SWDGE queues.
  2. Segment-sum via one-hot selector matmuls: for each 128-edge chunk,
     selector[p, j] = (src[p] - window_base == j) built with a DVE is_equal
     against an iota row (one batched build per gather piece); PE accumulates
     selector.T @ gathered into a PSUM window of 128 nodes.  Mean = PSUM *
     (1/cnt) on evacuation.
  3. Linearity moves the weight matmuls out of the edge loop:
     mean(x[dst]) @ W_msg.  The edge-feature path collapses to
     Hn @ (emb_table @ W_edge) where Hn[n, d] = count(src=n, feat=d)/cnt[n]
     is a host-computed *index* matrix; biases ride along as extra Hn rows.
  4. Dense phase in feature-major layout (PE transposes), BatchNorm stats
     via a 2KB AllReduce; BN + ReLU applied feature-major on the scalar
     engine (per-partition scale/shift) — no node-major BN pass.
  5. Layer-1 output is PE-transposed to node-major bf16 and AllGathered to
     become layer 2's gather source.  The final output stays feature-major
     [D, NPC]; the host transposes during unsharding.

Edge path (gathered x, selectors) runs in bf16; dense path in float32r.
"""
import os
import sys

if "/opt/trn_rl_repo" not in sys.path:
    sys.path.insert(0, "/opt/trn_rl_repo")

import numpy as np

import concourse.bass as bass  # noqa: F401  (engine types referenced via nc)
import concourse.tile as tile
from concourse import bacc, mybir
from concourse import bass_utils
from concourse.bass_interp import get_hw_module

F32 = mybir.dt.float32
F32R = mybir.dt.float32r
I16 = mybir.dt.int16
BF16 = mybir.dt.bfloat16
ALU = mybir.AluOpType
ACTF = mybir.ActivationFunctionType

EPS = 1e-5
B, S, D = 32, 512, 256
N = B * S                # 16384 nodes
DS, DD = 64, 64          # embedding table: [DS, DD]
L = 2                    # layers
NCORE = 8
NPC = N // NCORE         # 2048 nodes per core
WIN = 128                # nodes per PSUM window
NWIN = NPC // WIN        # 16 windows per core
NWING = N // WIN         # 128 windows globally
_CACHE = {}


def _pick_piece(epc):
    """Largest 128-multiple piece that divides epc and fits the SWDGE ring
    (descs/engine = piece/16 + 1 <= 128 -> piece <= 2032)."""
    for piece in range(1024, 512, -128):
        if epc % piece == 0:
            return piece
    return 512


def _balance(cnt):
    """Assign nodes to (window, pos) so per-window edge load is near-uniform.

    Greedy LPT over the 128 global windows (16 per core, 128 nodes each).
    Returns perm (node -> global position) and nch (chunks per window).
    Index-only computation.
    """
    import heapq
    order = np.argsort(-cnt, kind="stable")
    fill = np.zeros(NWING, np.int64)
    loads = np.zeros(NWING, np.int64)
    perm = np.empty(N, np.int64)
    heap = [(0, w) for w in range(NWING)]
    heapq.heapify(heap)
    for n in order:
        while True:
            load, w = heapq.heappop(heap)
            if fill[w] < WIN:
                break
        perm[n] = w * WIN + fill[w]
        fill[w] += 1
        loads[w] = load + cnt[n]
        if fill[w] < WIN:
            heapq.heappush(heap, (int(loads[w]), w))
    nch = max(int(np.ceil(loads.max() / 128)), 1)
    return perm, nch


def _build(nch, piece_src):
    """Build + schedule + bacc-compile the SPMD program.

    nch: chunks (of 128 edges) per 128-node window, uniform across cores
    (host pads every window to nch*128 edges).
    piece_src: per gather piece, 0 if every slot's dst lands in the first
    AllGather half (so the piece may start after AG#0), else 1.
    """
    cap = nch * WIN                  # padded edges per window
    epc = NWIN * cap                 # padded edges per core
    PIECE = _pick_piece(epc)         # edges per dma_gather call
    npiece = epc // PIECE            # gather calls per layer
    assert epc % PIECE == 0 and nch >= 2
    assert len(piece_src) == npiece
    NPCH = PIECE // 128              # chunks per piece

    nc = bacc.Bacc("TRN2", target_bir_lowering=False, debug=False,
                   num_devices=NCORE, num_swdge_queues=4)

    # ---- I/O ----
    x0 = nc.dram_tensor("x0", [N, D], BF16, kind="ExternalInput")
    xT0 = nc.dram_tensor("xT0", [D, NPC], F32R, kind="ExternalInput")
    idx_in = nc.dram_tensor("idx_in", [128, epc // 16], I16, kind="ExternalInput")
    srcmod_in = nc.dram_tensor("srcmod_in", [128, NWIN * nch], BF16, kind="ExternalInput")
    recip_in = nc.dram_tensor("recip_in", [128, NWIN], F32, kind="ExternalInput")
    hnt_in = nc.dram_tensor("hnt_in", [128, NPC], F32R, kind="ExternalInput")
    iota_in = nc.dram_tensor("iota_in", [128, 128], BF16, kind="ExternalInput")
    ident_in = nc.dram_tensor("ident_in", [128, 128], F32, kind="ExternalInput")
    identr_in = nc.dram_tensor("identr_in", [128, 128], F32R, kind="ExternalInput")
    wm_in = [nc.dram_tensor(f"wm{l}", [D, D], F32R, kind="ExternalInput") for l in range(L)]
    ws_in = [nc.dram_tensor(f"ws{l}", [D, D], F32R, kind="ExternalInput") for l in range(L)]
    embT_in = nc.dram_tensor("embT", [DD, DS], F32R, kind="ExternalInput")
    we_in = [nc.dram_tensor(f"we{l}", [DD, D], F32R, kind="ExternalInput") for l in range(L)]
    # rows DD..127 of the EW lhsT: [bm, be, bs, zeros...] packed on host
    ewc_in = [nc.dram_tensor(f"ewc{l}", [128 - DD, D], F32R, kind="ExternalInput")
              for l in range(L)]
    gam_in = [nc.dram_tensor(f"gam{l}", [D, 1], F32, kind="ExternalInput") for l in range(L)]
    bet_in = [nc.dram_tensor(f"bet{l}", [D, 1], F32, kind="ExternalInput") for l in range(L)]
    out_ext = nc.dram_tensor("out", [D, NPC], F32, kind="ExternalOutput")

    with tile.TileContext(nc) as tc:
        with tc.tile_pool(name="const", bufs=1) as cp, \
             tc.tile_pool(name="gat", bufs=8) as gp, \
             tc.tile_pool(name="selp", bufs=8) as sp, \
             tc.tile_pool(name="xmaj", bufs=1) as xp, \
             tc.tile_pool(name="psE", bufs=4, space="PSUM") as psE, \
             tc.tile_pool(name="psT", bufs=2, space="PSUM") as psT, \
             tc.tile_pool(name="psD", bufs=2, space="PSUM") as psD, \
             tc.tile_pool(name="dram", bufs=1, space="DRAM") as dp:

            # ---- constants into SBUF ----
            # Gather/selector gating inputs load first so the edge phase
            # starts as early as possible.
            NA_PIECES = 4                    # pieces preloaded first
            CPP = PIECE // 16                # idx columns per piece
            NSPLIT = NA_PIECES * CPP
            idx_a = cp.tile([128, NSPLIT], I16)
            idx_b = cp.tile([128, epc // 16 - NSPLIT], I16)
            srcmod = cp.tile([128, NWIN * nch], BF16)
            recip = cp.tile([128, NWIN], F32)
            hnt = cp.tile([128, NPC], F32R)
            iota = cp.tile([128, 128], BF16)
            ident = cp.tile([128, 128], F32)
            identr = cp.tile([128, 128], F32R)
            nc.sync.dma_start(out=idx_a[:, :], in_=idx_in[:, 0:NSPLIT])
            nc.sync.dma_start(out=iota[:, :], in_=iota_in[:, :])
            nc.sync.dma_start(out=srcmod[:, :], in_=srcmod_in[:, :])
            nc.sync.dma_start(out=idx_b[:, :], in_=idx_in[:, NSPLIT:])
            nc.sync.dma_start(out=recip[:, :], in_=recip_in[:, :])
            nc.sync.dma_start(out=ident[:, :], in_=ident_in[:, :])
            nc.sync.dma_start(out=identr[:, :], in_=identr_in[:, :])
            # big constants ride other engines' HWDGE queues so they don't
            # delay the idx/selector loads gating the first gathers
            nc.scalar.dma_start(out=hnt[:, :], in_=hnt_in[:, :])

            wm_sb, ws_sb, ew_sb = [], [], []
            embT_sb = cp.tile([DD, DS], F32R)
            nc.sync.dma_start(out=embT_sb[:, :], in_=embT_in[:, :])
            for l in range(L):
                wm = cp.tile([128, 2, D], F32R, name=f"wm_sb{l}")
                ws = cp.tile([128, 2, D], F32R, name=f"ws_sb{l}")
                for kt in range(2):
                    nc.scalar.dma_start(out=wm[:, kt, :], in_=wm_in[l][kt * 128:(kt + 1) * 128, :])
                    nc.scalar.dma_start(out=ws[:, kt, :], in_=ws_in[l][kt * 128:(kt + 1) * 128, :])
                wm_sb.append(wm)
                ws_sb.append(ws)

                we = cp.tile([DD, D], F32R, name=f"we_sb{l}")
                nc.sync.dma_start(out=we[:, :], in_=we_in[l][:, :])
                psew = psT.tile([DD, D], F32, tag="pst", name=f"psew{l}")
                nc.tensor.matmul(psew[:, :], embT_sb[:, :], we[:, :], start=True, stop=True)
                ew = cp.tile([128, D], F32R, name=f"ew_sb{l}")
                nc.vector.tensor_copy(ew[0:DD, :], psew[:, :])
                nc.sync.dma_start(out=ew[DD:128, :], in_=ewc_in[l][:, :])
                ew_sb.append(ew)

            gb_sb = []  # [128, 2] gamma / beta per layer, packed per feat-half
            for l in range(L):
                gam = cp.tile([128, 2], F32, name=f"gam_sb{l}")
                bet = cp.tile([128, 2], F32, name=f"bet_sb{l}")
                for f in range(2):
                    nc.sync.dma_start(out=gam[:, f:f + 1], in_=gam_in[l][f * 128:(f + 1) * 128, :])
                    nc.sync.dma_start(out=bet[:, f:f + 1], in_=bet_in[l][f * 128:(f + 1) * 128, :])
                gb_sb.append((gam, bet))

            xT_cur = [cp.tile([128, NPC], F32R, name=f"xT0_sb{f}") for f in range(2)]
            for f in range(2):
                nc.scalar.dma_start(out=xT_cur[f][:, :], in_=xT0[f * 128:(f + 1) * 128, :])

            # absorb one-time collective setup cost under the edge phase
            warm_sb = cp.tile([128, 1], F32, name="warm_sb")
            nc.vector.memset(warm_sb[:, :], 0.0)
            warm_in = dp.tile([128, 1], F32, name="warm_in")
            warm_out = dp.tile([128, 1], F32, addr_space="Shared", name="warm_out")
            nc.sync.dma_start(out=warm_in[:, :], in_=warm_sb[:, :])
            nc.gpsimd.collective_compute(
                "AllReduce", ALU.add,
                replica_groups=[list(range(NCORE))],
                ins=[warm_in[:, :]], outs=[warm_out[:, :]])
            warm_bk = cp.tile([128, 1], F32, name="warm_bk")
            nc.sync.dma_start(out=warm_bk[:, :], in_=warm_out[:, :])

            xsrc = x0  # gather source (DRAM AP-able handle)

            for l in range(L):
                # ================= edge phase =================
                pieces = [None] * npiece

                def ensure_piece(p, l=l, pieces=pieces):
                    if pieces[p] is not None:
                        return pieces[p]
                    g = gp.tile([128, NPCH, D], BF16, tag="g", name=f"g{l}_{p}")
                    idx_ap = (idx_a[:, p * CPP:(p + 1) * CPP]
                              if p < NA_PIECES else
                              idx_b[:, (p - NA_PIECES) * CPP:
                                       (p - NA_PIECES + 1) * CPP])
                    nc.gpsimd.dma_gather(
                        out_ap=g[:, :, :],
                        in_ap=xsrc[:, :],
                        idxs_ap=idx_ap,
                        num_idxs=PIECE, num_idxs_reg=PIECE,
                        elem_size=D, single_packet=True,
                        queue_num=p % 4)
                    s = sp.tile([128, NPCH, 128], BF16, tag="s", name=f"s{l}_{p}")
                    nc.vector.tensor_tensor(
                        s[:, :, :],
                        iota[:, :].unsqueeze(1).to_broadcast((128, NPCH, 128)),
                        srcmod[:, p * NPCH:(p + 1) * NPCH].unsqueeze(2)
                              .to_broadcast((128, NPCH, 128)),
                        ALU.is_equal)
                    pieces[p] = (g, s)
                    return pieces[p]

                # Fused edge + dense pipeline: windows stream through; after
                # every 4th window the corresponding 512-node dense block,
                # its stat partials, and its feat-major transposes fire.
                msx = xp.tile([128, NWIN, D], F32, tag="msx", name=f"msx{l}")
                preout = [xp.tile([128, NPC], F32, tag=f"pre{f}", name=f"pre{l}_{f}")
                          for f in range(2)]
                redp = cp.tile([128, 16], F32, tag="redp", bufs=2, name=f"redp{l}")
                sqscr = xp.tile([128, 512], F32, tag="sqscr", name=f"sqscr{l}")
                msxTn = [[None] * (NPC // 512) for _ in range(2)]
                for w in range(NWIN):
                    ps = psE.tile([128, D], F32, tag="pse", name=f"pse{l}_{w}")
                    for c in range(nch):
                        gc = w * nch + c
                        g, s = ensure_piece(gc // NPCH)
                        lc = gc % NPCH
                        nc.tensor.matmul(ps[:, :], s[:, lc, :], g[:, lc, :],
                                         start=(c == 0), stop=(c == nch - 1))
                    nc.vector.tensor_scalar(msx[:, w, :], ps[:, :],
                                            recip[:, w:w + 1], None, ALU.mult)
                    nb, wi = w // 4, w % 4
                    for f in range(2):
                        if wi == 0:
                            msxTn[f][nb] = xp.tile([128, 512], F32R, tag=f"msxT{f}",
                                                   bufs=4, name=f"msxT{l}_{f}_{nb}")
                        pt = psT.tile([128, 128], F32, tag="pst", name=f"ptm{l}_{w}_{f}")
                        nc.tensor.transpose(pt[:, :], msx[:, w, f * 128:(f + 1) * 128],
                                            ident[:, :])
                        dstp = msxTn[f][nb][:, wi * 128:(wi + 1) * 128]
                        if f == 0:
                            nc.vector.tensor_copy(dstp, pt[:, :])
                        else:
                            nc.scalar.activation(dstp, pt[:, :], ACTF.Copy,
                                                 bias=0.0, scale=1.0)
                    if wi != 3:
                        continue
                    # dense block for this group of 4 windows
                    cols = slice(nb * 512, (nb + 1) * 512)
                    for f in range(2):
                        pd = psD.tile([128, 512], F32, tag="psd", name=f"pd{l}_{f}_{nb}")
                        fo = slice(f * 128, (f + 1) * 128)
                        nc.tensor.matmul(pd[:, :], wm_sb[l][:, 0, fo], msxTn[0][nb][:, :],
                                         start=True, stop=False)
                        nc.tensor.matmul(pd[:, :], wm_sb[l][:, 1, fo], msxTn[1][nb][:, :],
                                         start=False, stop=False)
                        nc.tensor.matmul(pd[:, :], ws_sb[l][:, 0, fo], xT_cur[0][:, cols],
                                         start=False, stop=False)
                        nc.tensor.matmul(pd[:, :], ws_sb[l][:, 1, fo], xT_cur[1][:, cols],
                                         start=False, stop=False)
                        nc.tensor.matmul(pd[:, :], ew_sb[l][:, fo], hnt[:, cols],
                                         start=False, stop=True)
                        # evacuate + free per-block column sums
                        nc.vector.tensor_scalar(preout[f][:, cols], pd[:, :],
                                                1.0, 0.0, ALU.mult, ALU.add,
                                                accum_out=redp[:, f * 4 + nb:f * 4 + nb + 1])
                        # per-block sum of squares on the scalar engine
                        nc.scalar.activation(sqscr[:, :], preout[f][:, cols],
                                             ACTF.Square, bias=0.0, scale=1.0,
                                             accum_out=redp[:, 8 + f * 4 + nb:
                                                            9 + f * 4 + nb])

                # ================= batchnorm stats =================
                red = cp.tile([128, 4], F32, tag="red", bufs=2, name=f"red{l}")
                for f in range(2):
                    nc.vector.tensor_reduce(red[:, f:f + 1], redp[:, f * 4:(f + 1) * 4],
                                            mybir.AxisListType.X, ALU.add)
                    nc.vector.tensor_reduce(red[:, 2 + f:3 + f],
                                            redp[:, 8 + f * 4:8 + (f + 1) * 4],
                                            mybir.AxisListType.X, ALU.add)

                st_in = dp.tile([128, 4], F32, name=f"st_in{l}")
                st_out = dp.tile([128, 4], F32, addr_space="Shared", name=f"st_out{l}")
                nc.scalar.dma_start(out=st_in[:, :], in_=red[:, :])
                nc.gpsimd.collective_compute(
                    "AllReduce", ALU.add,
                    replica_groups=[list(range(NCORE))],
                    ins=[st_in[:, :]], outs=[st_out[:, :]])
                red2 = cp.tile([128, 4], F32, tag="red", bufs=2, name=f"red2{l}")
                nc.sync.dma_start(out=red2[:, :], in_=st_out[:, :])

                # mu/var -> scale/shift  (all [128, 2], column form)
                mo = cp.tile([128, 12], F32, tag="mo", bufs=2, name=f"mo{l}")
                mu, ex2, var, vare, sd, rsq = (mo[:, 0:2], mo[:, 2:4], mo[:, 4:6],
                                               mo[:, 6:8], mo[:, 8:10], mo[:, 10:12])
                nc.vector.tensor_scalar(mu, red2[:, 0:2], 1.0 / N, None, ALU.mult)
                nc.vector.tensor_scalar(ex2, red2[:, 2:4], 1.0 / N, None, ALU.mult)
                nc.vector.tensor_tensor(var, mu, mu, ALU.mult)
                nc.vector.tensor_tensor(var, ex2, var, ALU.subtract)
                nc.vector.tensor_scalar(vare, var, EPS, None, ALU.add)
                nc.scalar.activation(sd, vare, ACTF.Sqrt, bias=0.0, scale=1.0)
                nc.vector.reciprocal(rsq, sd)
                gam, bet = gb_sb[l]
                sc = cp.tile([128, 4], F32, tag="sc", bufs=2, name=f"sc{l}")
                scale2, shift2 = sc[:, 0:2], sc[:, 2:4]
                nc.vector.tensor_tensor(scale2, gam[:, :], rsq, ALU.mult)
                nc.vector.tensor_tensor(shift2, mu, scale2, ALU.mult)
                nc.vector.tensor_tensor(shift2, bet[:, :], shift2, ALU.subtract)

                # ===== BN + ReLU feature-major on the scalar engine =====
                xnT = [xp.tile([128, NPC], F32R if l < L - 1 else F32,
                               tag=f"xnT{f}", name=f"xnT{l}_{f}")
                       for f in range(2)]
                for f in range(2):
                    nc.scalar.activation(xnT[f][:, :], preout[f][:, :], ACTF.Relu,
                                         bias=shift2[:, f:f + 1],
                                         scale=scale2[:, f:f + 1])

                if l == L - 1:
                    for f in range(2):
                        nc.sync.dma_start(out=out_ext[f * 128:(f + 1) * 128, :],
                                          in_=xnT[f][:, :])
                else:
                    # node-major bf16 copy for the next layer's gather source
                    agi = dp.tile([NPC, D], BF16, name=f"agi{l}")
                    ago = dp.tile([N, D], BF16, addr_space="Shared", name=f"ago{l}")
                    xrow = xp.tile([128, NWIN, D], BF16, tag="xrow", name=f"xrow{l}")
                    for w in range(NWIN):
                        for f in range(2):
                            pt = psT.tile([128, 128], F32R, tag="pst",
                                          name=f"ptx{l}_{w}_{f}")
                            nc.tensor.transpose(pt[:, :],
                                                xnT[f][:, w * 128:(w + 1) * 128],
                                                identr[:, :])
                            dstp = xrow[:, w, f * 128:(f + 1) * 128]
                            if f == 0:
                                nc.vector.tensor_copy(dstp, pt[:, :])
                            else:
                                nc.scalar.activation(dstp, pt[:, :], ACTF.Copy,
                                                     bias=0.0, scale=1.0)
                    dst_ap = agi[:, :].rearrange("(w p) d -> p w d", p=128)
                    H = NWIN // 2
                    for h in range(2):
                        ws_ = slice(h * H, (h + 1) * H)
                        nc.sync.dma_start(out=dst_ap[:, ws_, :], in_=xrow[:, ws_, :])
                    nc.gpsimd.collective_compute(
                        "AllGather", ALU.bypass,
                        replica_groups=[list(range(NCORE))],
                        ins=[agi[:, :]], outs=[ago[:, :]])
                    xsrc = ago
                    xT_cur = xnT

    nc.compile()
    nc.m = get_hw_module(nc.m)
    return nc


def _preprocess(data, edge, edge_feature):
    """Host-side index preprocessing: balance nodes over windows, sort edges
    by (permuted) src, window-pad, build count matrices.  Touches only index
    arrays (+ dtype/layout of data)."""
    src = np.asarray(edge[0], dtype=np.int64)
    dst = np.asarray(edge[1], dtype=np.int64)
    ef = np.asarray(edge_feature, dtype=np.int64)

    cnt0 = np.bincount(src, minlength=N)
    perm, nch = _balance(cnt0)
    invperm = np.empty(N, np.int64)
    invperm[perm] = np.arange(N)

    psrc = perm[src]
    pdst = perm[dst]
    order = np.argsort(psrc, kind="stable")
    src_s = psrc[order]
    dst_s = pdst[order]

    cnt = np.bincount(psrc, minlength=N)
    recip = (1.0 / np.maximum(cnt, 1)).astype(np.float32)
    H = np.bincount(psrc * DS + ef, minlength=N * DS).reshape(N, DS)
    Hn = (H * recip[:, None]).astype(np.float32)

    cap = nch * WIN
    wcnt = np.bincount(src_s // WIN, minlength=NWING)
    assert wcnt.max() <= cap

    wstart = np.zeros(NWING + 1, np.int64)
    np.cumsum(wcnt, out=wstart[1:])
    idx_pad = np.zeros((NWING, cap), np.int16)   # layer-1 idx (x0 rows)
    idx2_pad = np.zeros((NWING, cap), np.int16)  # layer-2 idx (ago layout)
    srm_pad = np.full((NWING, cap), -1.0, np.float32)
    CH = NPC // 2
    kk_pad = np.zeros((NWING, cap), np.int8)     # AllGather half of each slot
    # ago position of node position p: half-major, then core, then offset
    pco = np.arange(N)
    ago_pos = ((pco % NPC) // CH) * (N // 2) + (pco // NPC) * CH + (pco % CH)
    for g in range(NWING):
        a, b = wstart[g], wstart[g + 1]
        k = b - a
        dsts = dst_s[a:b]
        ordh = np.argsort((dsts % NPC) // CH, kind="stable")
        dsts = dsts[ordh]
        idx_pad[g, :k] = dsts.astype(np.int16)
        idx2_pad[g, :k] = ago_pos[dsts].astype(np.int16)
        srm_pad[g, :k] = (src_s[a:b][ordh] - g * WIN).astype(np.float32)
        kk_pad[g, :k] = ((dsts % NPC) // CH).astype(np.int8)

    piece = _pick_piece(NWIN * cap)
    npiece = NWIN * cap // piece
    piece_src = np.zeros(npiece, np.int8)
    for w in range(NCORE):
        flat_kk = kk_pad[w * NWIN:(w + 1) * NWIN].reshape(-1)
        np.maximum(piece_src, flat_kk.reshape(npiece, piece).max(axis=1),
                   out=piece_src)
    piece_src = tuple(int(v) for v in piece_src)

    data2 = data.reshape(N, D)
    per_core = []
    for w in range(NCORE):
        gsl = slice(w * NWIN, (w + 1) * NWIN)
        nsl = slice(w * NPC, (w + 1) * NPC)
        orig = invperm[nsl]                           # original node ids
        flat_idx = idx_pad[gsl].reshape(-1)           # [NWIN*cap]
        idx_tile = np.tile(flat_idx.reshape(-1, 16).T, (8, 1)).astype(np.int16)

        srcmod = srm_pad[gsl].reshape(-1, 128).T.copy()      # [128, NWIN*nch]
        recip_sw = recip[nsl].reshape(NWIN, 128).T.copy()    # [128, NWIN]
        hnt = np.zeros((128, NPC), np.float32)
        hnt[:DS, :] = Hn[nsl].T
        nz = (cnt[nsl] > 0).astype(np.float32)
        hnt[DS, :] = nz
        hnt[DS + 1, :] = nz
        hnt[DS + 2, :] = 1.0
        xT0 = np.ascontiguousarray(data2[orig].T.astype(np.float32))
        import ml_dtypes as _md
        per_core.append(dict(idx_in=idx_tile,
                             srcmod_in=srcmod.astype(_md.bfloat16),
                             recip_in=recip_sw, hnt_in=hnt, xT0=xT0))
    return nch, perm, invperm, piece_src, per_core


def kernel(data, emb_table, W_msg, b_msg, W_self, b_self, W_edge, b_edge,
           bn_gamma, bn_beta, edge, edge_feature):
    data = np.asarray(data)
    nch, perm, invperm, piece_src, per_core = _preprocess(
        data, np.asarray(edge), np.asarray(edge_feature))

    key = (nch, piece_src)
    if key not in _CACHE:
        _CACHE[key] = _build(nch, piece_src)
    nc = _CACHE[key]

    import ml_dtypes
    x0 = np.ascontiguousarray(
        data.reshape(N, D)[invperm].astype(ml_dtypes.bfloat16))
    iota = np.broadcast_to(np.arange(128), (128, 128)).astype(ml_dtypes.bfloat16)
    ident = np.eye(128, dtype=np.float32)
    common = {
        "x0": x0, "iota_in": iota, "ident_in": ident, "identr_in": ident,
        "embT": np.ascontiguousarray(np.asarray(emb_table, np.float32).T),
    }
    for l in range(L):
        common[f"wm{l}"] = np.ascontiguousarray(np.asarray(W_msg[l], np.float32))
        common[f"ws{l}"] = np.ascontiguousarray(np.asarray(W_self[l], np.float32))
        common[f"we{l}"] = np.ascontiguousarray(np.asarray(W_edge[l], np.float32))
        ewc = np.zeros((128 - DD, D), np.float32)
        ewc[0] = np.asarray(b_msg[l], np.float32)
        ewc[1] = np.asarray(b_edge[l], np.float32)
        ewc[2] = np.asarray(b_self[l], np.float32)
        common[f"ewc{l}"] = ewc
        common[f"gam{l}"] = np.asarray(bn_gamma[l], np.float32).reshape(D, 1)
        common[f"bet{l}"] = np.asarray(bn_beta[l], np.float32).reshape(D, 1)

    in_maps = [{**common, **pc} for pc in per_core]
    trace = bool(os.environ.get("GNN_TRN_TRACE"))
    res = bass_utils.run_bass_kernel_spmd(
        nc, in_maps, core_ids=list(range(NCORE)), trace=trace)
    if trace:
        global LAST_RESULT
        LAST_RESULT = res
    out = np.concatenate([res.results[c]["out"] for c in range(NCORE)], axis=1)
    return np.ascontiguousarray(out.T[perm]).reshape(B, S, D).astype(np.float32)


LAST_RESULT = None


# revision 50
# speedup vs baseline: 1.4157x; 1.0195x over previous
"""EntityAggrNet (2-layer GNN message passing) on 8 Trainium2 NeuronCores.

Strategy
--------
Node-parallel sharding with host-side load balancing: nodes are assigned to
the 128 global PSUM windows (16 per core) by greedy LPT on edge count, so
every window carries ~E/128 edges (nch=33 chunks vs 34 unbalanced) and all
cores finish the edge phase together (small BN-stats AllReduce wait).
Edges are sorted by (balanced) src window on the host.

Per layer, per core:
  1. dma_gather x[dst] rows (bf16, 512B rows) from a replicated HBM copy of
     the layer input, 1024 edges per gather call, spread over 4 SWDGE queues.
  2. Segment-sum via one-hot selector matmuls: for each 128-edge chunk,
     selector[p, j] = (src[p] - window_base == j) built with a DVE is_equal
     against an iota row (one batched build per gather piece); PE accumulates
     selector.T @ gathered into a PSUM window of 128 nodes.  Mean = PSUM *
     (1/cnt) on evacuation.
  3. Linearity moves the weight matmuls out of the edge loop:
     mean(x[dst]) @ W_msg.  The edge-feature path collapses to
     Hn @ (emb_table @ W_edge) where Hn[n, d] = count(src=n, feat=d)/cnt[n]
     is a host-computed *index* matrix; biases ride along as extra Hn rows.
  4. Dense phase in feature-major layout (PE transposes), BatchNorm stats
     via a 2KB AllReduce; BN + ReLU applied feature-major on the scalar
     engine (per-partition scale/shift) — no node-major BN pass.
  5. Layer-1 output is PE-transposed to node-major bf16 and AllGathered to
     become layer 2's gather source.  The final output stays feature-major
     [D, NPC]; the host transposes during unsharding.

Edge path (gathered x, selectors) runs in bf16; dense path in float32r.
"""
import os
import sys

if "/opt/trn_rl_repo" not in sys.path:
    sys.path.insert(0, "/opt/trn_rl_repo")

import numpy as np

import concourse.bass as bass  # noqa: F401  (engine types referenced via nc)
import concourse.tile as tile
from concourse import bacc, mybir
from concourse import bass_utils
from concourse.bass_interp import get_hw_module

F32 = mybir.dt.float32
F32R = mybir.dt.float32r
I16 = mybir.dt.int16
BF16 = mybir.dt.bfloat16
ALU = mybir.AluOpType
ACTF = mybir.ActivationFunctionType

EPS = 1e-5
B, S, D = 32, 512, 256
N = B * S                # 16384 nodes
DS, DD = 64, 64          # embedding table: [DS, DD]
L = 2                    # layers
NCORE = 8
NPC = N // NCORE         # 2048 nodes per core
WIN = 128                # nodes per PSUM window
NWIN = NPC // WIN        # 16 windows per core
NWING = N // WIN         # 128 windows globally
_CACHE = {}


def _pick_piece(epc):
    """Largest 128-multiple piece that divides epc and fits the SWDGE ring
    (descs/engine = piece/16 + 1 <= 128 -> piece <= 2032)."""
    for piece in range(1024, 512, -128):
        if epc % piece == 0:
            return piece
    return 512


def _balance(cnt):
    """Assign nodes to (window, pos) so per-window edge load is near-uniform.

    Greedy LPT over the 128 global windows (16 per core, 128 nodes each).
    Returns perm (node -> global position) and nch (chunks per window).
    Index-only computation.
    """
    import heapq
    order = np.argsort(-cnt, kind="stable")
    fill = np.zeros(NWING, np.int64)
    loads = np.zeros(NWING, np.int64)
    perm = np.empty(N, np.int64)
    heap = [(0, w) for w in range(NWING)]
    heapq.heapify(heap)
    for n in order:
        while True:
            load, w = heapq.heappop(heap)
            if fill[w] < WIN:
                break
        perm[n] = w * WIN + fill[w]
        fill[w] += 1
        loads[w] = load + cnt[n]
        if fill[w] < WIN:
            heapq.heappush(heap, (int(loads[w]), w))
    nch = max(int(np.ceil(loads.max() / 128)), 1)
    return perm, nch


def _build(nch, piece_src):
    """Build + schedule + bacc-compile the SPMD program.

    nch: chunks (of 128 edges) per 128-node window, uniform across cores
    (host pads every window to nch*128 edges).
    piece_src: per gather piece, 0 if every slot's dst lands in the first
    AllGather half (so the piece may start after AG#0), else 1.
    """
    cap = nch * WIN                  # padded edges per window
    epc = NWIN * cap                 # padded edges per core
    PIECE = _pick_piece(epc)         # edges per dma_gather call
    npiece = epc // PIECE            # gather calls per layer
    assert epc % PIECE == 0 and nch >= 2
    assert len(piece_src) == npiece
    NPCH = PIECE // 128              # chunks per piece

    nc = bacc.Bacc("TRN2", target_bir_lowering=False, debug=False,
                   num_devices=NCORE, num_swdge_queues=4)

    # ---- I/O ----
    x0 = nc.dram_tensor("x0", [N, D], BF16, kind="ExternalInput")
    xT0 = nc.dram_tensor("xT0", [D, NPC], F32R, kind="ExternalInput")
    idx_in = nc.dram_tensor("idx_in", [128, epc // 16], I16, kind="ExternalInput")
    srcmod_in = nc.dram_tensor("srcmod_in", [128, NWIN * nch], BF16, kind="ExternalInput")
    recip_in = nc.dram_tensor("recip_in", [128, NWIN], F32, kind="ExternalInput")
    hnt_in = nc.dram_tensor("hnt_in", [128, NPC], F32R, kind="ExternalInput")
    iota_in = nc.dram_tensor("iota_in", [128, 128], BF16, kind="ExternalInput")
    ident_in = nc.dram_tensor("ident_in", [128, 128], F32, kind="ExternalInput")
    identr_in = nc.dram_tensor("identr_in", [128, 128], F32R, kind="ExternalInput")
    wm_in = [nc.dram_tensor(f"wm{l}", [D, D], F32R, kind="ExternalInput") for l in range(L)]
    ws_in = [nc.dram_tensor(f"ws{l}", [D, D], F32R, kind="ExternalInput") for l in range(L)]
    embT_in = nc.dram_tensor("embT", [DD, DS], F32R, kind="ExternalInput")
    we_in = [nc.dram_tensor(f"we{l}", [DD, D], F32R, kind="ExternalInput") for l in range(L)]
    # rows DD..127 of the EW lhsT: [bm, be, bs, zeros...] packed on host
    ewc_in = [nc.dram_tensor(f"ewc{l}", [128 - DD, D], F32R, kind="ExternalInput")
              for l in range(L)]
    gam_in = [nc.dram_tensor(f"gam{l}", [D, 1], F32, kind="ExternalInput") for l in range(L)]
    bet_in = [nc.dram_tensor(f"bet{l}", [D, 1], F32, kind="ExternalInput") for l in range(L)]
    out_ext = nc.dram_tensor("out", [D, NPC], F32, kind="ExternalOutput")

    with tile.TileContext(nc) as tc:
        with tc.tile_pool(name="const", bufs=1) as cp, \
             tc.tile_pool(name="gat", bufs=10) as gp, \
             tc.tile_pool(name="selp", bufs=10) as sp, \
             tc.tile_pool(name="xmaj", bufs=1) as xp, \
             tc.tile_pool(name="psE", bufs=4, space="PSUM") as psE, \
             tc.tile_pool(name="psT", bufs=2, space="PSUM") as psT, \
             tc.tile_pool(name="psD", bufs=2, space="PSUM") as psD, \
             tc.tile_pool(name="dram", bufs=1, space="DRAM") as dp:

            # ---- constants into SBUF ----
            # Gather/selector gating inputs load first so the edge phase
            # starts as early as possible.
            NA_PIECES = 4                    # pieces preloaded first
            CPP = PIECE // 16                # idx columns per piece
            NSPLIT = NA_PIECES * CPP
            idx_a = cp.tile([128, NSPLIT], I16)
            idx_b = cp.tile([128, epc // 16 - NSPLIT], I16)
            srcmod = cp.tile([128, NWIN * nch], BF16)
            recip = cp.tile([128, NWIN], F32)
            hnt = cp.tile([128, NPC], F32R)
            iota = cp.tile([128, 128], BF16)
            ident = cp.tile([128, 128], F32)
            identr = cp.tile([128, 128], F32R)
            nc.sync.dma_start(out=idx_a[:, :], in_=idx_in[:, 0:NSPLIT])
            nc.sync.dma_start(out=iota[:, :], in_=iota_in[:, :])
            nc.sync.dma_start(out=srcmod[:, :], in_=srcmod_in[:, :])
            nc.sync.dma_start(out=idx_b[:, :], in_=idx_in[:, NSPLIT:])
            nc.sync.dma_start(out=recip[:, :], in_=recip_in[:, :])
            nc.sync.dma_start(out=ident[:, :], in_=ident_in[:, :])
            nc.sync.dma_start(out=identr[:, :], in_=identr_in[:, :])
            # big constants ride other engines' HWDGE queues so they don't
            # delay the idx/selector loads gating the first gathers
            nc.scalar.dma_start(out=hnt[:, :], in_=hnt_in[:, :])

            wm_sb, ws_sb, ew_sb = [], [], []
            embT_sb = cp.tile([DD, DS], F32R)
            nc.sync.dma_start(out=embT_sb[:, :], in_=embT_in[:, :])
            for l in range(L):
                wm = cp.tile([128, 2, D], F32R, name=f"wm_sb{l}")
                ws = cp.tile([128, 2, D], F32R, name=f"ws_sb{l}")
                for kt in range(2):
                    nc.scalar.dma_start(out=wm[:, kt, :], in_=wm_in[l][kt * 128:(kt + 1) * 128, :])
                    nc.scalar.dma_start(out=ws[:, kt, :], in_=ws_in[l][kt * 128:(kt + 1) * 128, :])
                wm_sb.append(wm)
                ws_sb.append(ws)

                we = cp.tile([DD, D], F32R, name=f"we_sb{l}")
                nc.sync.dma_start(out=we[:, :], in_=we_in[l][:, :])
                psew = psT.tile([DD, D], F32, tag="pst", name=f"psew{l}")
                nc.tensor.matmul(psew[:, :], embT_sb[:, :], we[:, :], start=True, stop=True)
                ew = cp.tile([128, D], F32R, name=f"ew_sb{l}")
                nc.vector.tensor_copy(ew[0:DD, :], psew[:, :])
                nc.sync.dma_start(out=ew[DD:128, :], in_=ewc_in[l][:, :])
                ew_sb.append(ew)

            gb_sb = []  # [128, 2] gamma / beta per layer, packed per feat-half
            for l in range(L):
                gam = cp.tile([128, 2], F32, name=f"gam_sb{l}")
                bet = cp.tile([128, 2], F32, name=f"bet_sb{l}")
                for f in range(2):
                    nc.sync.dma_start(out=gam[:, f:f + 1], in_=gam_in[l][f * 128:(f + 1) * 128, :])
                    nc.sync.dma_start(out=bet[:, f:f + 1], in_=bet_in[l][f * 128:(f + 1) * 128, :])
                gb_sb.append((gam, bet))

            xT_cur = [cp.tile([128, NPC], F32R, name=f"xT0_sb{f}") for f in range(2)]
            for f in range(2):
                nc.scalar.dma_start(out=xT_cur[f][:, :], in_=xT0[f * 128:(f + 1) * 128, :])

            # absorb one-time collective setup cost under the edge phase
            warm_sb = cp.tile([128, 1], F32, name="warm_sb")
            nc.vector.memset(warm_sb[:, :], 0.0)
            warm_in = dp.tile([128, 1], F32, name="warm_in")
            warm_out = dp.tile([128, 1], F32, addr_space="Shared", name="warm_out")
            nc.sync.dma_start(out=warm_in[:, :], in_=warm_sb[:, :])
            nc.gpsimd.collective_compute(
                "AllReduce", ALU.add,
                replica_groups=[list(range(NCORE))],
                ins=[warm_in[:, :]], outs=[warm_out[:, :]])
            warm_bk = cp.tile([128, 1], F32, name="warm_bk")
            nc.sync.dma_start(out=warm_bk[:, :], in_=warm_out[:, :])

            xsrc = x0  # gather source (DRAM AP-able handle)

            for l in range(L):
                # ================= edge phase =================
                pieces = [None] * npiece

                def ensure_piece(p, l=l, pieces=pieces):
                    if pieces[p] is not None:
                        return pieces[p]
                    g = gp.tile([128, NPCH, D], BF16, tag="g", name=f"g{l}_{p}")
                    idx_ap = (idx_a[:, p * CPP:(p + 1) * CPP]
                              if p < NA_PIECES else
                              idx_b[:, (p - NA_PIECES) * CPP:
                                       (p - NA_PIECES + 1) * CPP])
                    nc.gpsimd.dma_gather(
                        out_ap=g[:, :, :],
                        in_ap=xsrc[:, :],
                        idxs_ap=idx_ap,
                        num_idxs=PIECE, num_idxs_reg=PIECE,
                        elem_size=D, single_packet=True,
                        queue_num=p % 4)
                    s = sp.tile([128, NPCH, 128], BF16, tag="s", name=f"s{l}_{p}")
                    nc.vector.tensor_tensor(
                        s[:, :, :],
                        iota[:, :].unsqueeze(1).to_broadcast((128, NPCH, 128)),
                        srcmod[:, p * NPCH:(p + 1) * NPCH].unsqueeze(2)
                              .to_broadcast((128, NPCH, 128)),
                        ALU.is_equal)
                    pieces[p] = (g, s)
                    return pieces[p]

                # Fused edge + dense pipeline: windows stream through; after
                # every 4th window the corresponding 512-node dense block,
                # its stat partials, and its feat-major transposes fire.
                msx = xp.tile([128, NWIN, D], F32, tag="msx", name=f"msx{l}")
                preout = [xp.tile([128, NPC], F32, tag=f"pre{f}", name=f"pre{l}_{f}")
                          for f in range(2)]
                redp = cp.tile([128, 16], F32, tag="redp", bufs=2, name=f"redp{l}")
                sqscr = xp.tile([128, 512], F32, tag="sqscr", name=f"sqscr{l}")
                msxTn = [[None] * (NPC // 512) for _ in range(2)]
                for w in range(NWIN):
                    ps = psE.tile([128, D], F32, tag="pse", name=f"pse{l}_{w}")
                    for c in range(nch):
                        gc = w * nch + c
                        g, s = ensure_piece(gc // NPCH)
                        lc = gc % NPCH
                        nc.tensor.matmul(ps[:, :], s[:, lc, :], g[:, lc, :],
                                         start=(c == 0), stop=(c == nch - 1))
                    nc.vector.tensor_scalar(msx[:, w, :], ps[:, :],
                                            recip[:, w:w + 1], None, ALU.mult)
                    nb, wi = w // 4, w % 4
                    for f in range(2):
                        if wi == 0:
                            msxTn[f][nb] = xp.tile([128, 512], F32R, tag=f"msxT{f}",
                                                   bufs=4, name=f"msxT{l}_{f}_{nb}")
                        pt = psT.tile([128, 128], F32, tag="pst", name=f"ptm{l}_{w}_{f}")
                        nc.tensor.transpose(pt[:, :], msx[:, w, f * 128:(f + 1) * 128],
                                            ident[:, :])
                        dstp = msxTn[f][nb][:, wi * 128:(wi + 1) * 128]
                        if f == 0:
                            nc.vector.tensor_copy(dstp, pt[:, :])
                        else:
                            nc.scalar.activation(dstp, pt[:, :], ACTF.Copy,
                                                 bias=0.0, scale=1.0)
                    if wi != 3:
                        continue
                    # dense block for this group of 4 windows
                    cols = slice(nb * 512, (nb + 1) * 512)
                    for f in range(2):
                        pd = psD.tile([128, 512], F32, tag="psd", name=f"pd{l}_{f}_{nb}")
                        fo = slice(f * 128, (f + 1) * 128)
                        nc.tensor.matmul(pd[:, :], wm_sb[l][:, 0, fo], msxTn[0][nb][:, :],
                                         start=True, stop=False)
                        nc.tensor.matmul(pd[:, :], wm_sb[l][:, 1, fo], msxTn[1][nb][:, :],
                                         start=False, stop=False)
                        nc.tensor.matmul(pd[:, :], ws_sb[l][:, 0, fo], xT_cur[0][:, cols],
                                         start=False, stop=False)
                        nc.tensor.matmul(pd[:, :], ws_sb[l][:, 1, fo], xT_cur[1][:, cols],
                                         start=False, stop=False)
                        nc.tensor.matmul(pd[:, :], ew_sb[l][:, fo], hnt[:, cols],
                                         start=False, stop=True)
                        # evacuate + free per-block column sums
                        nc.vector.tensor_scalar(preout[f][:, cols], pd[:, :],
                                                1.0, 0.0, ALU.mult, ALU.add,
                                                accum_out=redp[:, f * 4 + nb:f * 4 + nb + 1])
                        # per-block sum of squares on the scalar engine
                        nc.scalar.activation(sqscr[:, :], preout[f][:, cols],
                                             ACTF.Square, bias=0.0, scale=1.0,
                                             accum_out=redp[:, 8 + f * 4 + nb:
                                                            9 + f * 4 + nb])

                # ================= batchnorm stats =================
                red = cp.tile([128, 4], F32, tag="red", bufs=2, name=f"red{l}")
                for f in range(2):
                    nc.vector.tensor_reduce(red[:, f:f + 1], redp[:, f * 4:(f + 1) * 4],
                                            mybir.AxisListType.X, ALU.add)
                    nc.vector.tensor_reduce(red[:, 2 + f:3 + f],
                                            redp[:, 8 + f * 4:8 + (f + 1) * 4],
                                            mybir.AxisListType.X, ALU.add)

                st_in = dp.tile([128, 4], F32, name=f"st_in{l}")
                st_out = dp.tile([128, 4], F32, addr_space="Shared", name=f"st_out{l}")
                nc.scalar.dma_start(out=st_in[:, :], in_=red[:, :])
                nc.gpsimd.collective_compute(
                    "AllReduce", ALU.add,
                    replica_groups=[list(range(NCORE))],
                    ins=[st_in[:, :]], outs=[st_out[:, :]])
                red2 = cp.tile([128, 4], F32, tag="red", bufs=2, name=f"red2{l}")
                nc.sync.dma_start(out=red2[:, :], in_=st_out[:, :])

                # mu/var -> scale/shift  (all [128, 2], column form)
                mo = cp.tile([128, 12], F32, tag="mo", bufs=2, name=f"mo{l}")
                mu, ex2, var, vare, sd, rsq = (mo[:, 0:2], mo[:, 2:4], mo[:, 4:6],
                                               mo[:, 6:8], mo[:, 8:10], mo[:, 10:12])
                nc.vector.tensor_scalar(mu, red2[:, 0:2], 1.0 / N, None, ALU.mult)
                nc.vector.tensor_scalar(ex2, red2[:, 2:4], 1.0 / N, None, ALU.mult)
                nc.vector.tensor_tensor(var, mu, mu, ALU.mult)
                nc.vector.tensor_tensor(var, ex2, var, ALU.subtract)
                nc.vector.tensor_scalar(vare, var, EPS, None, ALU.add)
                nc.scalar.activation(sd, vare, ACTF.Sqrt, bias=0.0, scale=1.0)
                nc.vector.reciprocal(rsq, sd)
                gam, bet = gb_sb[l]
                sc = cp.tile([128, 4], F32, tag="sc", bufs=2, name=f"sc{l}")
                scale2, shift2 = sc[:, 0:2], sc[:, 2:4]
                nc.vector.tensor_tensor(scale2, gam[:, :], rsq, ALU.mult)
                nc.vector.tensor_tensor(shift2, mu, scale2, ALU.mult)
                nc.vector.tensor_tensor(shift2, bet[:, :], shift2, ALU.subtract)

                # ===== BN + ReLU feature-major on the scalar engine =====
                xnT = [xp.tile([128, NPC], F32R if l < L - 1 else F32,
                               tag=f"xnT{f}", name=f"xnT{l}_{f}")
                       for f in range(2)]
                for f in range(2):
                    nc.scalar.activation(xnT[f][:, :], preout[f][:, :], ACTF.Relu,
                                         bias=shift2[:, f:f + 1],
                                         scale=scale2[:, f:f + 1])

                if l == L - 1:
                    for f in range(2):
                        nc.sync.dma_start(out=out_ext[f * 128:(f + 1) * 128, :],
                                          in_=xnT[f][:, :])
                else:
                    # node-major bf16 copy for the next layer's gather source
                    agi = dp.tile([NPC, D], BF16, name=f"agi{l}")
                    ago = dp.tile([N, D], BF16, addr_space="Shared", name=f"ago{l}")
                    xrow = xp.tile([128, NWIN, D], BF16, tag="xrow", name=f"xrow{l}")
                    for w in range(NWIN):
                        for f in range(2):
                            pt = psT.tile([128, 128], F32R, tag="pst",
                                          name=f"ptx{l}_{w}_{f}")
                            nc.tensor.transpose(pt[:, :],
                                                xnT[f][:, w * 128:(w + 1) * 128],
                                                identr[:, :])
                            dstp = xrow[:, w, f * 128:(f + 1) * 128]
                            if f == 0:
                                nc.vector.tensor_copy(dstp, pt[:, :])
                            else:
                                nc.scalar.activation(dstp, pt[:, :], ACTF.Copy,
                                                     bias=0.0, scale=1.0)
                    dst_ap = agi[:, :].rearrange("(w p) d -> p w d", p=128)
                    H = NWIN // 2
                    for h in range(2):
                        ws_ = slice(h * H, (h + 1) * H)
                        nc.sync.dma_start(out=dst_ap[:, ws_, :], in_=xrow[:, ws_, :])
                    nc.gpsimd.collective_compute(
                        "AllGather", ALU.bypass,
                        replica_groups=[list(range(NCORE))],
                        ins=[agi[:, :]], outs=[ago[:, :]])
                    xsrc = ago
                    xT_cur = xnT

    nc.compile()
    nc.m = get_hw_module(nc.m)
    return nc


def _preprocess(data, edge, edge_feature):
    """Host-side index preprocessing: balance nodes over windows, sort edges
    by (permuted) src, window-pad, build count matrices.  Touches only index
    arrays (+ dtype/layout of data)."""
    src = np.asarray(edge[0], dtype=np.int64)
    dst = np.asarray(edge[1], dtype=np.int64)
    ef = np.asarray(edge_feature, dtype=np.int64)

    cnt0 = np.bincount(src, minlength=N)
    perm, nch = _balance(cnt0)
    invperm = np.empty(N, np.int64)
    invperm[perm] = np.arange(N)

    psrc = perm[src]
    pdst = perm[dst]
    order = np.argsort(psrc, kind="stable")
    src_s = psrc[order]
    dst_s = pdst[order]

    cnt = np.bincount(psrc, minlength=N)
    recip = (1.0 / np.maximum(cnt, 1)).astype(np.float32)
    H = np.bincount(psrc * DS + ef, minlength=N * DS).reshape(N, DS)
    Hn = (H * recip[:, None]).astype(np.float32)

    cap = nch * WIN
    wcnt = np.bincount(src_s // WIN, minlength=NWING)
    assert wcnt.max() <= cap

    wstart = np.zeros(NWING + 1, np.int64)
    np.cumsum(wcnt, out=wstart[1:])
    idx_pad = np.zeros((NWING, cap), np.int16)   # layer-1 idx (x0 rows)
    idx2_pad = np.zeros((NWING, cap), np.int16)  # layer-2 idx (ago layout)
    srm_pad = np.full((NWING, cap), -1.0, np.float32)
    CH = NPC // 2
    kk_pad = np.zeros((NWING, cap), np.int8)     # AllGather half of each slot
    # ago position of node position p: half-major, then core, then offset
    pco = np.arange(N)
    ago_pos = ((pco % NPC) // CH) * (N // 2) + (pco // NPC) * CH + (pco % CH)
    for g in range(NWING):
        a, b = wstart[g], wstart[g + 1]
        k = b - a
        dsts = dst_s[a:b]
        ordh = np.argsort((dsts % NPC) // CH, kind="stable")
        dsts = dsts[ordh]
        idx_pad[g, :k] = dsts.astype(np.int16)
        idx2_pad[g, :k] = ago_pos[dsts].astype(np.int16)
        srm_pad[g, :k] = (src_s[a:b][ordh] - g * WIN).astype(np.float32)
        kk_pad[g, :k] = ((dsts % NPC) // CH).astype(np.int8)

    piece = _pick_piece(NWIN * cap)
    npiece = NWIN * cap // piece
    piece_src = np.zeros(npiece, np.int8)
    for w in range(NCORE):
        flat_kk = kk_pad[w * NWIN:(w + 1) * NWIN].reshape(-1)
        np.maximum(piece_src, flat_kk.reshape(npiece, piece).max(axis=1),
                   out=piece_src)
    piece_src = tuple(int(v) for v in piece_src)

    data2 = data.reshape(N, D)
    per_core = []
    for w in range(NCORE):
        gsl = slice(w * NWIN, (w + 1) * NWIN)
        nsl = slice(w * NPC, (w + 1) * NPC)
        orig = invperm[nsl]                           # original node ids
        flat_idx = idx_pad[gsl].reshape(-1)           # [NWIN*cap]
        idx_tile = np.tile(flat_idx.reshape(-1, 16).T, (8, 1)).astype(np.int16)

        srcmod = srm_pad[gsl].reshape(-1, 128).T.copy()      # [128, NWIN*nch]
        recip_sw = recip[nsl].reshape(NWIN, 128).T.copy()    # [128, NWIN]
        hnt = np.zeros((128, NPC), np.float32)
        hnt[:DS, :] = Hn[nsl].T
        nz = (cnt[nsl] > 0).astype(np.float32)
        hnt[DS, :] = nz
        hnt[DS + 1, :] = nz
        hnt[DS + 2, :] = 1.0
        xT0 = np.ascontiguousarray(data2[orig].T.astype(np.float32))
        import ml_dtypes as _md
        per_core.append(dict(idx_in=idx_tile,
                             srcmod_in=srcmod.astype(_md.bfloat16),
                             recip_in=recip_sw, hnt_in=hnt, xT0=xT0))
    return nch, perm, invperm, piece_src, per_core


def kernel(data, emb_table, W_msg, b_msg, W_self, b_self, W_edge, b_edge,
           bn_gamma, bn_beta, edge, edge_feature):
    data = np.asarray(data)
    nch, perm, invperm, piece_src, per_core = _preprocess(
        data, np.asarray(edge), np.asarray(edge_feature))

    key = (nch, piece_src)
    if key not in _CACHE:
        _CACHE[key] = _build(nch, piece_src)
    nc = _CACHE[key]

    import ml_dtypes
    x0 = np.ascontiguousarray(
        data.reshape(N, D)[invperm].astype(ml_dtypes.bfloat16))
    iota = np.broadcast_to(np.arange(128), (128, 128)).astype(ml_dtypes.bfloat16)
    ident = np.eye(128, dtype=np.float32)
    common = {
        "x0": x0, "iota_in": iota, "ident_in": ident, "identr_in": ident,
        "embT": np.ascontiguousarray(np.asarray(emb_table, np.float32).T),
    }
    for l in range(L):
        common[f"wm{l}"] = np.ascontiguousarray(np.asarray(W_msg[l], np.float32))
        common[f"ws{l}"] = np.ascontiguousarray(np.asarray(W_self[l], np.float32))
        common[f"we{l}"] = np.ascontiguousarray(np.asarray(W_edge[l], np.float32))
        ewc = np.zeros((128 - DD, D), np.float32)
        ewc[0] = np.asarray(b_msg[l], np.float32)
        ewc[1] = np.asarray(b_edge[l], np.float32)
        ewc[2] = np.asarray(b_self[l], np.float32)
        common[f"ewc{l}"] = ewc
        common[f"gam{l}"] = np.asarray(bn_gamma[l], np.float32).reshape(D, 1)
        common[f"bet{l}"] = np.asarray(bn_beta[l], np.float32).reshape(D, 1)

    in_maps = [{**common, **pc} for pc in per_core]
    trace = bool(os.environ.get("GNN_TRN_TRACE"))
    res = bass_utils.run_bass_kernel_spmd(
        nc, in_maps, core_ids=list(range(NCORE)), trace=trace)
    if trace:
        global LAST_RESULT
        LAST_RESULT = res
    out = np.concatenate([res.results[c]["out"] for c in range(NCORE)], axis=1)
    return np.ascontiguousarray(out.T[perm]).reshape(B, S, D).astype(np.float32)


LAST_RESULT = None


# revision 56
# speedup vs baseline: 1.4474x; 1.0224x over previous
"""EntityAggrNet (2-layer GNN message passing) on 8 Trainium2 NeuronCores.

Strategy
--------
Node-parallel sharding with host-side load balancing: nodes are assigned to
the 128 global PSUM windows (16 per core) by greedy LPT on edge count, so
every window carries ~E/128 edges (nch=33 chunks vs 34 unbalanced) and all
cores finish the edge phase together (small BN-stats AllReduce wait).
Edges are sorted by (balanced) src window on the host.

Per layer, per core:
  1. dma_gather x[dst] rows (bf16, 512B rows) from a replicated HBM copy of
     the layer input, 1024 edges per gather call, spread over 4 SWDGE queues.
  2. Segment-sum via one-hot selector matmuls: for each 128-edge chunk,
     selector[p, j] = (src[p] - window_base == j) built with a DVE is_equal
     against an iota row (one batched build per gather piece); PE accumulates
     selector.T @ gathered into a PSUM window of 128 nodes.  Mean = PSUM *
     (1/cnt) on evacuation.
  3. Linearity moves the weight matmuls out of the edge loop:
     mean(x[dst]) @ W_msg.  The edge-feature path collapses to
     Hn @ (emb_table @ W_edge) where Hn[n, d] = count(src=n, feat=d)/cnt[n]
     is a host-computed *index* matrix; biases ride along as extra Hn rows.
  4. Dense phase in feature-major layout (PE transposes), BatchNorm stats
     via a 2KB AllReduce; BN + ReLU applied feature-major on the scalar
     engine (per-partition scale/shift) — no node-major BN pass.
  5. Layer-1 output is PE-transposed to node-major bf16 and AllGathered to
     become layer 2's gather source.  The final output stays feature-major
     [D, NPC]; the host transposes during unsharding.

Edge path (gathered x, selectors) runs in bf16; dense path in float32r.
"""
import os
import sys

if "/opt/trn_rl_repo" not in sys.path:
    sys.path.insert(0, "/opt/trn_rl_repo")

import numpy as np

import concourse.bass as bass  # noqa: F401  (engine types referenced via nc)
import concourse.tile as tile
from concourse import bacc, mybir
from concourse import bass_utils
from concourse.bass_interp import get_hw_module

F32 = mybir.dt.float32
F32R = mybir.dt.float32r
I16 = mybir.dt.int16
BF16 = mybir.dt.bfloat16
ALU = mybir.AluOpType
ACTF = mybir.ActivationFunctionType

EPS = 1e-5
B, S, D = 32, 512, 256
N = B * S                # 16384 nodes
DS, DD = 64, 64          # embedding table: [DS, DD]
L = 2                    # layers
NCORE = 8
NPC = N // NCORE         # 2048 nodes per core
WIN = 128                # nodes per PSUM window
NWIN = NPC // WIN        # 16 windows per core
NWING = N // WIN         # 128 windows globally
_CACHE = {}


def _pick_piece(epc):
    """Largest 128-multiple piece that divides epc and fits the SWDGE ring
    (descs/engine = piece/16 + 1 <= 128 -> piece <= 2032)."""
    for piece in range(1024, 512, -128):
        if epc % piece == 0:
            return piece
    return 512


def _balance(cnt):
    """Assign nodes to (window, pos) so per-window edge load is near-uniform.

    Greedy LPT over the 128 global windows (16 per core, 128 nodes each).
    Returns perm (node -> global position) and nch (chunks per window).
    Index-only computation.
    """
    import heapq
    order = np.argsort(-cnt, kind="stable")
    fill = np.zeros(NWING, np.int64)
    loads = np.zeros(NWING, np.int64)
    perm = np.empty(N, np.int64)
    heap = [(0, w) for w in range(NWING)]
    heapq.heapify(heap)
    for n in order:
        while True:
            load, w = heapq.heappop(heap)
            if fill[w] < WIN:
                break
        perm[n] = w * WIN + fill[w]
        fill[w] += 1
        loads[w] = load + cnt[n]
        if fill[w] < WIN:
            heapq.heappush(heap, (int(loads[w]), w))
    nch = max(int(np.ceil(loads.max() / 128)), 1)
    return perm, nch


def _build(nch, piece_src):
    """Build + schedule + bacc-compile the SPMD program.

    nch: chunks (of 128 edges) per 128-node window, uniform across cores
    (host pads every window to nch*128 edges).
    piece_src: per gather piece, 0 if every slot's dst lands in the first
    AllGather half (so the piece may start after AG#0), else 1.
    """
    cap = nch * WIN                  # padded edges per window
    epc = NWIN * cap                 # padded edges per core
    PIECE = _pick_piece(epc)         # edges per dma_gather call
    npiece = epc // PIECE            # gather calls per layer
    assert epc % PIECE == 0 and nch >= 2
    assert len(piece_src) == npiece
    NPCH = PIECE // 128              # chunks per piece

    nc = bacc.Bacc("TRN2", target_bir_lowering=False, debug=False,
                   num_devices=NCORE, num_swdge_queues=4)

    # ---- I/O ----
    x0 = nc.dram_tensor("x0", [N, D], BF16, kind="ExternalInput")
    xT0 = nc.dram_tensor("xT0", [D, NPC], F32R, kind="ExternalInput")
    idx_in = nc.dram_tensor("idx_in", [128, epc // 16], I16, kind="ExternalInput")
    srcmod_in = nc.dram_tensor("srcmod_in", [128, NWIN * nch], BF16, kind="ExternalInput")
    recip_in = nc.dram_tensor("recip_in", [128, NWIN], F32, kind="ExternalInput")
    hnt_in = nc.dram_tensor("hnt_in", [128, NPC], F32R, kind="ExternalInput")
    iota_in = nc.dram_tensor("iota_in", [128, 128], BF16, kind="ExternalInput")
    ident_in = nc.dram_tensor("ident_in", [128, 128], F32, kind="ExternalInput")
    identr_in = nc.dram_tensor("identr_in", [128, 128], F32R, kind="ExternalInput")
    wm_in = [nc.dram_tensor(f"wm{l}", [D, D], F32R, kind="ExternalInput") for l in range(L)]
    ws_in = [nc.dram_tensor(f"ws{l}", [D, D], F32R, kind="ExternalInput") for l in range(L)]
    embT_in = nc.dram_tensor("embT", [DD, DS], F32R, kind="ExternalInput")
    we_in = [nc.dram_tensor(f"we{l}", [DD, D], F32R, kind="ExternalInput") for l in range(L)]
    # rows DD..127 of the EW lhsT: [bm, be, bs, zeros...] packed on host
    ewc_in = [nc.dram_tensor(f"ewc{l}", [128 - DD, D], F32R, kind="ExternalInput")
              for l in range(L)]
    gam_in = [nc.dram_tensor(f"gam{l}", [D, 1], F32, kind="ExternalInput") for l in range(L)]
    bet_in = [nc.dram_tensor(f"bet{l}", [D, 1], F32, kind="ExternalInput") for l in range(L)]
    out_ext = nc.dram_tensor("out", [D, NPC], F32, kind="ExternalOutput")

    with tile.TileContext(nc) as tc:
        with tc.tile_pool(name="const", bufs=1) as cp, \
             tc.tile_pool(name="gat", bufs=10) as gp, \
             tc.tile_pool(name="selp", bufs=10) as sp, \
             tc.tile_pool(name="xmaj", bufs=1) as xp, \
             tc.tile_pool(name="psE", bufs=4, space="PSUM") as psE, \
             tc.tile_pool(name="psT", bufs=2, space="PSUM") as psT, \
             tc.tile_pool(name="psD", bufs=2, space="PSUM") as psD, \
             tc.tile_pool(name="dram", bufs=1, space="DRAM") as dp:

            # ---- constants into SBUF ----
            # Gather/selector gating inputs load first so the edge phase
            # starts as early as possible.
            NA_PIECES = 4                    # pieces preloaded first
            CPP = PIECE // 16                # idx columns per piece
            NSPLIT = NA_PIECES * CPP
            idx_a = cp.tile([128, NSPLIT], I16)
            idx_b = cp.tile([128, epc // 16 - NSPLIT], I16)
            srcmod = cp.tile([128, NWIN * nch], BF16)
            recip = cp.tile([128, NWIN], F32)
            hnt = cp.tile([128, NPC], F32R)
            iota = cp.tile([128, 128], BF16)
            ident = cp.tile([128, 128], F32)
            identr = cp.tile([128, 128], F32R)
            ones1 = cp.tile([1, 128], F32)
            nc.vector.memset(ones1[:, :], 1.0)
            nc.sync.dma_start(out=idx_a[:, :], in_=idx_in[:, 0:NSPLIT])
            nc.sync.dma_start(out=iota[:, :], in_=iota_in[:, :])
            nc.sync.dma_start(out=srcmod[:, :], in_=srcmod_in[:, :])
            nc.sync.dma_start(out=idx_b[:, :], in_=idx_in[:, NSPLIT:])
            nc.sync.dma_start(out=recip[:, :], in_=recip_in[:, :])
            nc.sync.dma_start(out=ident[:, :], in_=ident_in[:, :])
            nc.sync.dma_start(out=identr[:, :], in_=identr_in[:, :])
            # big constants ride other engines' HWDGE queues so they don't
            # delay the idx/selector loads gating the first gathers
            nc.scalar.dma_start(out=hnt[:, :], in_=hnt_in[:, :])

            wm_sb, ws_sb, ew_sb = [], [], []
            embT_sb = cp.tile([DD, DS], F32R)
            nc.sync.dma_start(out=embT_sb[:, :], in_=embT_in[:, :])
            for l in range(L):
                wm = cp.tile([128, 2, D], F32R, name=f"wm_sb{l}")
                ws = cp.tile([128, 2, D], F32R, name=f"ws_sb{l}")
                for kt in range(2):
                    nc.scalar.dma_start(out=wm[:, kt, :], in_=wm_in[l][kt * 128:(kt + 1) * 128, :])
                    nc.scalar.dma_start(out=ws[:, kt, :], in_=ws_in[l][kt * 128:(kt + 1) * 128, :])
                wm_sb.append(wm)
                ws_sb.append(ws)

                we = cp.tile([DD, D], F32R, name=f"we_sb{l}")
                nc.sync.dma_start(out=we[:, :], in_=we_in[l][:, :])
                psew = psT.tile([DD, D], F32, tag="pst", name=f"psew{l}")
                nc.tensor.matmul(psew[:, :], embT_sb[:, :], we[:, :], start=True, stop=True)
                ew = cp.tile([128, D], F32R, name=f"ew_sb{l}")
                nc.vector.tensor_copy(ew[0:DD, :], psew[:, :])
                nc.sync.dma_start(out=ew[DD:128, :], in_=ewc_in[l][:, :])
                ew_sb.append(ew)

            gb_sb = []  # [128, 2] gamma / beta per layer, packed per feat-half
            for l in range(L):
                gam = cp.tile([128, 2], F32, name=f"gam_sb{l}")
                bet = cp.tile([128, 2], F32, name=f"bet_sb{l}")
                for f in range(2):
                    nc.sync.dma_start(out=gam[:, f:f + 1], in_=gam_in[l][f * 128:(f + 1) * 128, :])
                    nc.sync.dma_start(out=bet[:, f:f + 1], in_=bet_in[l][f * 128:(f + 1) * 128, :])
                gb_sb.append((gam, bet))

            xT_cur = [cp.tile([128, NPC], F32R, name=f"xT0_sb{f}") for f in range(2)]
            for f in range(2):
                nc.scalar.dma_start(out=xT_cur[f][:, :], in_=xT0[f * 128:(f + 1) * 128, :])

            # absorb one-time collective setup cost under the edge phase
            warm_sb = cp.tile([128, 1], F32, name="warm_sb")
            nc.vector.memset(warm_sb[:, :], 0.0)
            warm_in = dp.tile([128, 1], F32, name="warm_in")
            warm_out = dp.tile([128, 1], F32, addr_space="Shared", name="warm_out")
            nc.sync.dma_start(out=warm_in[:, :], in_=warm_sb[:, :])
            nc.gpsimd.collective_compute(
                "AllReduce", ALU.add,
                replica_groups=[list(range(NCORE))],
                ins=[warm_in[:, :]], outs=[warm_out[:, :]])
            warm_bk = cp.tile([128, 1], F32, name="warm_bk")
            nc.sync.dma_start(out=warm_bk[:, :], in_=warm_out[:, :])

            xsrc = x0  # gather source (DRAM AP-able handle)

            for l in range(L):
                # ================= edge phase =================
                pieces = [None] * npiece

                def ensure_piece(p, l=l, pieces=pieces):
                    if pieces[p] is not None:
                        return pieces[p]
                    g = gp.tile([128, NPCH, D], BF16, tag="g", name=f"g{l}_{p}")
                    idx_ap = (idx_a[:, p * CPP:(p + 1) * CPP]
                              if p < NA_PIECES else
                              idx_b[:, (p - NA_PIECES) * CPP:
                                       (p - NA_PIECES + 1) * CPP])
                    nc.gpsimd.dma_gather(
                        out_ap=g[:, :, :],
                        in_ap=xsrc[:, :],
                        idxs_ap=idx_ap,
                        num_idxs=PIECE, num_idxs_reg=PIECE,
                        elem_size=D, single_packet=True,
                        queue_num=p % 4)
                    s = sp.tile([128, NPCH, 128], BF16, tag="s", name=f"s{l}_{p}")
                    nc.vector.tensor_tensor(
                        s[:, :, :],
                        iota[:, :].unsqueeze(1).to_broadcast((128, NPCH, 128)),
                        srcmod[:, p * NPCH:(p + 1) * NPCH].unsqueeze(2)
                              .to_broadcast((128, NPCH, 128)),
                        ALU.is_equal)
                    pieces[p] = (g, s)
                    return pieces[p]

                # Fused edge + dense pipeline: windows stream through; after
                # every 4th window the corresponding 512-node dense block,
                # its stat partials, and its feat-major transposes fire.
                msx = xp.tile([128, NWIN, D], F32, tag="msx", name=f"msx{l}")
                preout = [xp.tile([128, NPC], F32, tag=f"pre{f}", name=f"pre{l}_{f}")
                          for f in range(2)]
                redp = cp.tile([128, 16], F32, tag="redp", bufs=2, name=f"redp{l}")
                sqscr = xp.tile([128, 512], F32, tag="sqscr", name=f"sqscr{l}")
                msxTn = [[None] * (NPC // 512) for _ in range(2)]
                if l < L - 1:
                    xraw = xp.tile([128, NWIN, D], F32, tag="xraw", name=f"xraw{l}")
                for w in range(NWIN):
                    ps = psE.tile([128, D], F32, tag="pse", name=f"pse{l}_{w}")
                    for c in range(nch):
                        gc = w * nch + c
                        g, s = ensure_piece(gc // NPCH)
                        lc = gc % NPCH
                        nc.tensor.matmul(ps[:, :], s[:, lc, :], g[:, lc, :],
                                         start=(c == 0), stop=(c == nch - 1))
                    nc.vector.tensor_scalar(msx[:, w, :], ps[:, :],
                                            recip[:, w:w + 1], None, ALU.mult)
                    nb, wi = w // 4, w % 4
                    for f in range(2):
                        if wi == 0:
                            msxTn[f][nb] = xp.tile([128, 512], F32R, tag=f"msxT{f}",
                                                   bufs=4, name=f"msxT{l}_{f}_{nb}")
                        pt = psT.tile([128, 128], F32, tag="pst", name=f"ptm{l}_{w}_{f}")
                        nc.tensor.transpose(pt[:, :], msx[:, w, f * 128:(f + 1) * 128],
                                            ident[:, :])
                        dstp = msxTn[f][nb][:, wi * 128:(wi + 1) * 128]
                        if f == 0:
                            nc.vector.tensor_copy(dstp, pt[:, :])
                        else:
                            nc.scalar.activation(dstp, pt[:, :], ACTF.Copy,
                                                 bias=0.0, scale=1.0)
                    if wi != 3:
                        continue
                    # dense block for this group of 4 windows
                    cols = slice(nb * 512, (nb + 1) * 512)
                    for f in range(2):
                        pd = psD.tile([128, 512], F32, tag="psd", name=f"pd{l}_{f}_{nb}")
                        fo = slice(f * 128, (f + 1) * 128)
                        nc.tensor.matmul(pd[:, :], wm_sb[l][:, 0, fo], msxTn[0][nb][:, :],
                                         start=True, stop=False)
                        nc.tensor.matmul(pd[:, :], wm_sb[l][:, 1, fo], msxTn[1][nb][:, :],
                                         start=False, stop=False)
                        nc.tensor.matmul(pd[:, :], ws_sb[l][:, 0, fo], xT_cur[0][:, cols],
                                         start=False, stop=False)
                        nc.tensor.matmul(pd[:, :], ws_sb[l][:, 1, fo], xT_cur[1][:, cols],
                                         start=False, stop=False)
                        nc.tensor.matmul(pd[:, :], ew_sb[l][:, fo], hnt[:, cols],
                                         start=False, stop=True)
                        # evacuate + free per-block column sums
                        nc.vector.tensor_scalar(preout[f][:, cols], pd[:, :],
                                                1.0, 0.0, ALU.mult, ALU.add,
                                                accum_out=redp[:, f * 4 + nb:f * 4 + nb + 1])
                        # per-block sum of squares on the scalar engine
                        nc.scalar.activation(sqscr[:, :], preout[f][:, cols],
                                             ACTF.Square, bias=0.0, scale=1.0,
                                             accum_out=redp[:, 8 + f * 4 + nb:
                                                            9 + f * 4 + nb])
                    if l < L - 1:
                        # pre-BN node-major transposes, hidden under the edge
                        # phase; BN applies node-major after the stats AR
                        for w2 in range(nb * 4, nb * 4 + 4):
                            for f in range(2):
                                pt = psT.tile([128, 128], F32, tag="pst",
                                              name=f"ptx{l}_{w2}_{f}")
                                nc.tensor.transpose(
                                    pt[:, :], preout[f][:, w2 * 128:(w2 + 1) * 128],
                                    ident[:, :])
                                dstp = xraw[:, w2, f * 128:(f + 1) * 128]
                                if f == 0:
                                    nc.vector.tensor_copy(dstp, pt[:, :])
                                else:
                                    nc.scalar.activation(dstp, pt[:, :], ACTF.Copy,
                                                         bias=0.0, scale=1.0)

                # ================= batchnorm stats =================
                red = cp.tile([128, 4], F32, tag="red", bufs=2, name=f"red{l}")
                for f in range(2):
                    nc.vector.tensor_reduce(red[:, f:f + 1], redp[:, f * 4:(f + 1) * 4],
                                            mybir.AxisListType.X, ALU.add)
                    nc.vector.tensor_reduce(red[:, 2 + f:3 + f],
                                            redp[:, 8 + f * 4:8 + (f + 1) * 4],
                                            mybir.AxisListType.X, ALU.add)

                st_in = dp.tile([128, 4], F32, name=f"st_in{l}")
                st_out = dp.tile([128, 4], F32, addr_space="Shared", name=f"st_out{l}")
                nc.scalar.dma_start(out=st_in[:, :], in_=red[:, :])
                nc.gpsimd.collective_compute(
                    "AllReduce", ALU.add,
                    replica_groups=[list(range(NCORE))],
                    ins=[st_in[:, :]], outs=[st_out[:, :]])
                red2 = cp.tile([128, 4], F32, tag="red", bufs=2, name=f"red2{l}")
                nc.sync.dma_start(out=red2[:, :], in_=st_out[:, :])

                # mu/var -> scale/shift  (all [128, 2], column form)
                mo = cp.tile([128, 12], F32, tag="mo", bufs=2, name=f"mo{l}")
                mu, ex2, var, vare, sd, rsq = (mo[:, 0:2], mo[:, 2:4], mo[:, 4:6],
                                               mo[:, 6:8], mo[:, 8:10], mo[:, 10:12])
                nc.vector.tensor_scalar(mu, red2[:, 0:2], 1.0 / N, None, ALU.mult)
                nc.vector.tensor_scalar(ex2, red2[:, 2:4], 1.0 / N, None, ALU.mult)
                nc.vector.tensor_tensor(var, mu, mu, ALU.mult)
                nc.vector.tensor_tensor(var, ex2, var, ALU.subtract)
                nc.vector.tensor_scalar(vare, var, EPS, None, ALU.add)
                nc.scalar.activation(sd, vare, ACTF.Sqrt, bias=0.0, scale=1.0)
                nc.vector.reciprocal(rsq, sd)
                gam, bet = gb_sb[l]
                sc = cp.tile([128, 4], F32, tag="sc", bufs=2, name=f"sc{l}")
                scale2, shift2 = sc[:, 0:2], sc[:, 2:4]
                nc.vector.tensor_tensor(scale2, gam[:, :], rsq, ALU.mult)
                nc.vector.tensor_tensor(shift2, mu, scale2, ALU.mult)
                nc.vector.tensor_tensor(shift2, bet[:, :], shift2, ALU.subtract)

                # ===== BN + ReLU feature-major on the scalar engine =====
                xnT = [xp.tile([128, NPC], F32R if l < L - 1 else F32,
                               tag=f"xnT{f}", name=f"xnT{l}_{f}")
                       for f in range(2)]
                for f in range(2):
                    nc.scalar.activation(xnT[f][:, :], preout[f][:, :], ACTF.Relu,
                                         bias=shift2[:, f:f + 1],
                                         scale=scale2[:, f:f + 1])

                if l == L - 1:
                    for f in range(2):
                        nc.sync.dma_start(out=out_ext[f * 128:(f + 1) * 128, :],
                                          in_=xnT[f][:, :])
                else:
                    # broadcast scale/shift along partitions via PE, then BN +
                    # ReLU node-major on the pre-transposed xraw (short
                    # post-AllReduce chain: the transposes already happened
                    # under the edge phase)
                    scrow = cp.tile([1, 4, 128], F32, tag="scrow", bufs=2,
                                    name=f"scrow{l}")
                    for k in range(4):
                        psck = psT.tile([1, 128], F32, tag="pst",
                                        name=f"psc{l}_{k}")
                        nc.tensor.transpose(psck[:, :], sc[:, k:k + 1],
                                            ident[:, :])
                        nc.vector.tensor_copy(scrow[0:1, k, :], psck[:, :])
                    scb = cp.tile([128, 2, D], F32, tag="scb", bufs=2,
                                  name=f"scb{l}")
                    for j in range(2):  # j=0: scale, j=1: shift
                        pb = psT.tile([128, D], F32, tag="pst", name=f"pb{l}_{j}")
                        nc.tensor.matmul(pb[:, 0:128], ones1[:, :],
                                         scrow[0:1, 2 * j, :],
                                         start=True, stop=False)
                        nc.tensor.matmul(pb[:, 128:256], ones1[:, :],
                                         scrow[0:1, 2 * j + 1, :],
                                         start=False, stop=True)
                        nc.vector.tensor_copy(scb[:, j, :], pb[:, :])

                    agi = dp.tile([NPC, D], BF16, name=f"agi{l}")
                    ago = dp.tile([N, D], BF16, addr_space="Shared", name=f"ago{l}")
                    xrow = xp.tile([128, NWIN, D], BF16, tag="xrow", name=f"xrow{l}")
                    tmp = xp.tile([128, NWIN, D], F32, tag="msx", name=f"xtmp{l}")
                    dst_ap = agi[:, :].rearrange("(w p) d -> p w d", p=128)
                    H = NWIN // 2
                    for h in range(2):
                        ws_ = slice(h * H, (h + 1) * H)
                        bc = (128, H, D)
                        nc.vector.scalar_tensor_tensor(
                            tmp[:, ws_, :], xraw[:, ws_, :], 1.0,
                            scb[:, 0, :].unsqueeze(1).to_broadcast(bc),
                            ALU.mult, ALU.mult)
                        nc.vector.tensor_tensor(
                            tmp[:, ws_, :], tmp[:, ws_, :],
                            scb[:, 1, :].unsqueeze(1).to_broadcast(bc), ALU.add)
                        nc.vector.tensor_scalar(xrow[:, ws_, :], tmp[:, ws_, :],
                                                0.0, None, ALU.max)
                        nc.sync.dma_start(out=dst_ap[:, ws_, :], in_=xrow[:, ws_, :])
                    nc.gpsimd.collective_compute(
                        "AllGather", ALU.bypass,
                        replica_groups=[list(range(NCORE))],
                        ins=[agi[:, :]], outs=[ago[:, :]])
                    xsrc = ago
                    xT_cur = xnT

    nc.compile()
    nc.m = get_hw_module(nc.m)
    return nc


def _preprocess(data, edge, edge_feature):
    """Host-side index preprocessing: balance nodes over windows, sort edges
    by (permuted) src, window-pad, build count matrices.  Touches only index
    arrays (+ dtype/layout of data)."""
    src = np.asarray(edge[0], dtype=np.int64)
    dst = np.asarray(edge[1], dtype=np.int64)
    ef = np.asarray(edge_feature, dtype=np.int64)

    cnt0 = np.bincount(src, minlength=N)
    perm, nch = _balance(cnt0)
    invperm = np.empty(N, np.int64)
    invperm[perm] = np.arange(N)

    psrc = perm[src]
    pdst = perm[dst]
    order = np.argsort(psrc, kind="stable")
    src_s = psrc[order]
    dst_s = pdst[order]

    cnt = np.bincount(psrc, minlength=N)
    recip = (1.0 / np.maximum(cnt, 1)).astype(np.float32)
    H = np.bincount(psrc * DS + ef, minlength=N * DS).reshape(N, DS)
    Hn = (H * recip[:, None]).astype(np.float32)

    cap = nch * WIN
    wcnt = np.bincount(src_s // WIN, minlength=NWING)
    assert wcnt.max() <= cap

    wstart = np.zeros(NWING + 1, np.int64)
    np.cumsum(wcnt, out=wstart[1:])
    idx_pad = np.zeros((NWING, cap), np.int16)   # layer-1 idx (x0 rows)
    idx2_pad = np.zeros((NWING, cap), np.int16)  # layer-2 idx (ago layout)
    srm_pad = np.full((NWING, cap), -1.0, np.float32)
    CH = NPC // 2
    kk_pad = np.zeros((NWING, cap), np.int8)     # AllGather half of each slot
    # ago position of node position p: half-major, then core, then offset
    pco = np.arange(N)
    ago_pos = ((pco % NPC) // CH) * (N // 2) + (pco // NPC) * CH + (pco % CH)
    for g in range(NWING):
        a, b = wstart[g], wstart[g + 1]
        k = b - a
        dsts = dst_s[a:b]
        ordh = np.argsort((dsts % NPC) // CH, kind="stable")
        dsts = dsts[ordh]
        idx_pad[g, :k] = dsts.astype(np.int16)
        idx2_pad[g, :k] = ago_pos[dsts].astype(np.int16)
        srm_pad[g, :k] = (src_s[a:b][ordh] - g * WIN).astype(np.float32)
        kk_pad[g, :k] = ((dsts % NPC) // CH).astype(np.int8)

    piece = _pick_piece(NWIN * cap)
    npiece = NWIN * cap // piece
    piece_src = np.zeros(npiece, np.int8)
    for w in range(NCORE):
        flat_kk = kk_pad[w * NWIN:(w + 1) * NWIN].reshape(-1)
        np.maximum(piece_src, flat_kk.reshape(npiece, piece).max(axis=1),
                   out=piece_src)
    piece_src = tuple(int(v) for v in piece_src)

    data2 = data.reshape(N, D)
    per_core = []
    for w in range(NCORE):
        gsl = slice(w * NWIN, (w + 1) * NWIN)
        nsl = slice(w * NPC, (w + 1) * NPC)
        orig = invperm[nsl]                           # original node ids
        flat_idx = idx_pad[gsl].reshape(-1)           # [NWIN*cap]
        idx_tile = np.tile(flat_idx.reshape(-1, 16).T, (8, 1)).astype(np.int16)

        srcmod = srm_pad[gsl].reshape(-1, 128).T.copy()      # [128, NWIN*nch]
        recip_sw = recip[nsl].reshape(NWIN, 128).T.copy()    # [128, NWIN]
        hnt = np.zeros((128, NPC), np.float32)
        hnt[:DS, :] = Hn[nsl].T
        nz = (cnt[nsl] > 0).astype(np.float32)
        hnt[DS, :] = nz
        hnt[DS + 1, :] = nz
        hnt[DS + 2, :] = 1.0
        xT0 = np.ascontiguousarray(data2[orig].T.astype(np.float32))
        import ml_dtypes as _md
        per_core.append(dict(idx_in=idx_tile,
                             srcmod_in=srcmod.astype(_md.bfloat16),
                             recip_in=recip_sw, hnt_in=hnt, xT0=xT0))
    return nch, perm, invperm, piece_src, per_core


def kernel(data, emb_table, W_msg, b_msg, W_self, b_self, W_edge, b_edge,
           bn_gamma, bn_beta, edge, edge_feature):
    data = np.asarray(data)
    nch, perm, invperm, piece_src, per_core = _preprocess(
        data, np.asarray(edge), np.asarray(edge_feature))

    key = (nch, piece_src)
    if key not in _CACHE:
        _CACHE[key] = _build(nch, piece_src)
    nc = _CACHE[key]

    import ml_dtypes
    x0 = np.ascontiguousarray(
        data.reshape(N, D)[invperm].astype(ml_dtypes.bfloat16))
    iota = np.broadcast_to(np.arange(128), (128, 128)).astype(ml_dtypes.bfloat16)
    ident = np.eye(128, dtype=np.float32)
    common = {
        "x0": x0, "iota_in": iota, "ident_in": ident, "identr_in": ident,
        "embT": np.ascontiguousarray(np.asarray(emb_table, np.float32).T),
    }
    for l in range(L):
        common[f"wm{l}"] = np.ascontiguousarray(np.asarray(W_msg[l], np.float32))
        common[f"ws{l}"] = np.ascontiguousarray(np.asarray(W_self[l], np.float32))
        common[f"we{l}"] = np.ascontiguousarray(np.asarray(W_edge[l], np.float32))
        ewc = np.zeros((128 - DD, D), np.float32)
        ewc[0] = np.asarray(b_msg[l], np.float32)
        ewc[1] = np.asarray(b_edge[l], np.float32)
        ewc[2] = np.asarray(b_self[l], np.float32)
        common[f"ewc{l}"] = ewc
        common[f"gam{l}"] = np.asarray(bn_gamma[l], np.float32).reshape(D, 1)
        common[f"bet{l}"] = np.asarray(bn_beta[l], np.float32).reshape(D, 1)

    in_maps = [{**common, **pc} for pc in per_core]
    trace = bool(os.environ.get("GNN_TRN_TRACE"))
    res = bass_utils.run_bass_kernel_spmd(
        nc, in_maps, core_ids=list(range(NCORE)), trace=trace)
    if trace:
        global LAST_RESULT
        LAST_RESULT = res
    out = np.concatenate([res.results[c]["out"] for c in range(NCORE)], axis=1)
    return np.ascontiguousarray(out.T[perm]).reshape(B, S, D).astype(np.float32)


LAST_RESULT = None
